# revision 14
# baseline (speedup 1.0000x reference)
"""TimeSformer-style divided space-time attention block on 8 trn2 NeuronCores.

Sharding: core = 2*b + h (b in 0..3, h in 0..1).
  Launch 1 (temporal attn + fc + residual): core handles batch b, patch-half h
    (98 of 196 patches), all T=8 frames. 784 tokens/core in (n_loc, t) order.
  Launch 2 (spatial attn + cls + MLP):      core handles batch b, frame-half h
    (4 of 8 frames), all 196 patches + cls. Host reshuffles between launches.

Layouts: activations feature-major (C on partitions) for matmuls; LN/softmax
stats token-major via PE transposes; weights host-pre-transposed; q-scale
folded into wq/qb; softmax denominator via ones-column appended to V.
Matmuls in float32r (except fc2: bf16), everything else fp32, exact erf-GELU.
"""

import numpy as np
import ml_dtypes
from contextlib import ExitStack

import concourse.bass as bass
import concourse.tile as tile
from concourse import bacc, mybir
from concourse.bass_utils import run_bass_kernel_spmd

F32 = mybir.dt.float32
F32R = mybir.dt.float32r
BF16 = mybir.dt.bfloat16
AF = mybir.ActivationFunctionType
OP = mybir.AluOpType

B, T, N, C = 4, 8, 196, 1024
H, D = 16, 64
SCALE = D ** -0.5
HID = 4 * C
EPS = 1e-5
NH = N // 2            # 98 patches per core in launch 1
TOK1 = NH * T          # 784 tokens per core, launch 1
FPC = T // 2           # 4 frames per core in launch 2
TOK2 = 1 + FPC * N     # 785 tokens per core, launch 2 (cls + 4*196)
FLAT2 = FPC * 197      # 788: spatial qkv token list, cls duplicated per frame
CC = C // 128          # 8 chunks of C
HC = HID // 128        # 32 chunks of HID
VW = 66                # per-head V block: 64 V + ones col + pad (even N)
V66 = H * VW           # 1056
N_CORES = 8


def _tiles(total, step=128):
    return [(i, min(step, total - i)) for i in range(0, total, step)]


def _bcast_row(ap_1d, parts=128):
    """DRAM (n,) -> DMA-source AP broadcasting over `parts` partitions."""
    return bass.AP(tensor=ap_1d.tensor, offset=ap_1d.offset,
                   ap=[[0, parts]] + list(ap_1d.ap))


def _ln_to_fm(nc, tm_src_tile, tn, dst_fm, dst_cols, g_sb, b_sb,
              eps_sb, psT, tmp, ident_sb):
    """LN a token-major tile (tn<=128 tokens x C) and write normalized*g+b
    transposed into dst_fm[:, cc, ...] (feature-major, fp32r).
    dst_cols: list of (col0, length, src0) runs."""
    st = tmp.tile([128, 2, 6], F32, tag="st")
    nc.vector.bn_stats(st[:tn, 0], tm_src_tile[:tn, 0:512])
    nc.vector.bn_stats(st[:tn, 1], tm_src_tile[:tn, 512:1024])
    mv = tmp.tile([128, 2], F32, tag="mv")
    nc.vector.bn_aggr(mv[:tn], st[:tn])
    rstd = tmp.tile([128, 1], F32, tag="rstd")
    nc.scalar.activation(rstd[:tn], mv[:tn, 1:2], AF.Sqrt, bias=eps_sb[:tn])
    nc.vector.reciprocal(rstd[:tn], rstd[:tn])
    y = tmp.tile([128, C], F32, tag="y")
    nc.vector.tensor_scalar(y[:tn], tm_src_tile[:tn], mv[:tn, 0:1], rstd[:tn],
                            op0=OP.subtract, op1=OP.mult)
    for cc in range(CC):
        pt = psT.tile([128, 128], F32)
        nc.tensor.transpose(pt[:, :tn], y[:tn, cc * 128:(cc + 1) * 128],
                            ident_sb[:tn, :tn])
        for (col0, length, src0) in dst_cols:
            nc.scalar.activation(dst_fm[:, cc, col0:col0 + length],
                                 pt[:, src0:src0 + length], AF.Identity,
                                 bias=b_sb[:, cc:cc + 1], scale=g_sb[:, cc:cc + 1])


def build_launch1():
    nc = bacc.Bacc("TRN2", target_bir_lowering=False, debug=False,
                   enable_asserts=False, num_devices=N_CORES)

    def din(name, shape, dt=F32R):
        return nc.dram_tensor(name, shape, dt, kind="ExternalInput").ap()

    x_tm = din("x_tm", [TOK1, C], F32)
    x_fm = din("x_fm", [C, TOK1], F32)
    wq = din("wq_T", [CC, 128, C]); wk = din("wk_T", [CC, 128, C])
    wv = din("wv_T", [128, CC * V66])
    wp = din("wp_T", [CC, 128, C]); wfc = din("wfc_T", [CC, 128, C])
    # cblock: [0:128 ident][128:256 mask][256:1312 vb_bcast]
    #         [1312.. g,b,qb,bp,fcb cols (8 each)]
    cblock = din("cblock", [128, 1352], F32)
    xt_out = nc.dram_tensor("xt_out", [C, TOK1], F32, kind="ExternalOutput").ap()

    toks = _tiles(TOK1)  # 7 tiles: 6x128 + 16

    with tile.TileContext(nc) as tc, ExitStack() as ctx:
        const = ctx.enter_context(tc.tile_pool(name="const", bufs=1))
        cb = const.tile([128, 1352], F32)
        nc.sync.dma_start(cb, cblock)
        ident_sb = cb[:, 0:128]; mask_sb = cb[:, 128:256]
        vb_sb = cb[:, 256:256 + V66]
        g_sb = cb[:, 1312:1320]; b_sb = cb[:, 1320:1328]
        qb_sb = cb[:, 1328:1336]; bp_sb = cb[:, 1336:1344]
        fcb_sb = cb[:, 1344:1352]
        eps_sb = const.tile([128, 1], F32); nc.vector.memset(eps_sb, EPS)

        big = ctx.enter_context(tc.tile_pool(name="big", bufs=1))
        wpool = ctx.enter_context(tc.tile_pool(name="w", bufs=3))
        x_fm_sb = big.tile([128, CC, TOK1], F32, tag="x_fm")
        nc.sync.dma_start(x_fm_sb, x_fm.rearrange("(cc p) t -> p cc t", p=128))

        def load_w1024(wT, cc):
            w_sb = wpool.tile([128, CC, 128], F32R, tag="w1024")
            nc.sync.dma_start(w_sb, wT[cc].rearrange("p (kc j) -> p kc j", j=128))
            return w_sb

        # -------- stage A: LN + transpose -> x_lnT --------
        x_lnT = big.tile([128, CC, TOK1], F32R, tag="x_lnT")
        with tc.tile_pool(name="ln_tmp", bufs=2) as tmp, \
             tc.tile_pool(name="psA", bufs=4, space="PSUM") as psT:
            for (t0, tn) in toks:
                xt_t = tmp.tile([128, C], F32, tag="xt")
                nc.sync.dma_start(xt_t[:tn], x_tm[t0:t0 + tn])
                _ln_to_fm(nc, xt_t, tn, x_lnT, [(t0, tn, 0)],
                          g_sb, b_sb, eps_sb, psT, tmp, ident_sb)

        # -------- stage B: QKV --------
        q_fm = big.tile([128, CC, TOK1], F32R, tag="q")
        k_fm = big.tile([128, CC, TOK1], F32R, tag="k")
        with tc.tile_pool(name="psB", bufs=4, space="PSUM") as psQ:
            for (dst, wT, bias_cols) in ((q_fm, wq, qb_sb), (k_fm, wk, None)):
                for cc in range(CC):
                    w_sb = load_w1024(wT, cc)
                    for (t0, tlen) in ((0, 392), (392, 392)):
                        ps = psQ.tile([128, 392], F32)
                        for kc in range(CC):
                            nc.tensor.matmul(ps, w_sb[:, kc],
                                             x_lnT[:, kc, t0:t0 + tlen],
                                             start=(kc == 0), stop=(kc == CC - 1))
                        if bias_cols is None:
                            nc.vector.tensor_copy(dst[:, cc, t0:t0 + tlen], ps)
                        else:
                            nc.scalar.activation(dst[:, cc, t0:t0 + tlen], ps,
                                                 AF.Identity,
                                                 bias=bias_cols[:, cc:cc + 1])

        wv_sb = big.tile([128, CC, V66], F32R, tag="wv")
        nc.sync.dma_start(wv_sb, wv.rearrange("p (kc j) -> p kc j", j=V66))
        v_tm = big.tile([128, len(toks), V66], F32R, tag="v")
        with tc.tile_pool(name="psV", bufs=4, space="PSUM") as psV:
            for it, (t0, tn) in enumerate(toks):
                for (n0, nlen) in ((0, 512), (512, 512), (1024, 32)):
                    ps = psV.tile([128, 512], F32)
                    for kc in range(CC):
                        nc.tensor.matmul(ps[:tn, :nlen],
                                         x_lnT[:, kc, t0:t0 + tn],
                                         wv_sb[:, kc, n0:n0 + nlen],
                                         start=(kc == 0), stop=(kc == CC - 1))
                    nc.vector.tensor_add(v_tm[:tn, it, n0:n0 + nlen],
                                         ps[:tn, :nlen], vb_sb[:tn, n0:n0 + nlen])

        # -------- stage C: temporal attention (block-diag-8 within tile) -----
        o_tm = big.tile([128, len(toks), C], F32, tag="wv")  # reuses wv slot
        with tc.tile_pool(name="att", bufs=6) as att, \
             tc.tile_pool(name="psL", bufs=3, space="PSUM") as psL, \
             tc.tile_pool(name="psO", bufs=3, space="PSUM") as psO:
            for it, (t0, tn) in enumerate(toks):
                qn = min(256, TOK1 - t0)
                for hh in range(H):
                    ch = hh // 2; po = 64 * (hh % 2)
                    psl = psL.tile([128, 256], F32)
                    nc.tensor.matmul(psl[:tn, :qn],
                                     k_fm[po:po + 64, ch, t0:t0 + tn],
                                     q_fm[po:po + 64, ch, t0:t0 + qn],
                                     start=True, stop=True)
                    et = att.tile([128, 128], F32, tag="et")
                    nc.scalar.activation(et[:tn, :tn], psl[:tn, 0:tn], AF.Exp)
                    at = att.tile([128, 128], F32R, tag="at")
                    nc.vector.tensor_mul(at[:tn, :tn], et[:tn, :tn],
                                         mask_sb[:tn, :tn])
                    pso = psO.tile([128, 66], F32)
                    nc.tensor.matmul(pso[:tn], at[:tn, :tn],
                                     v_tm[:tn, it, hh * VW:hh * VW + 66],
                                     start=True, stop=True)
                    rt = att.tile([128, 1], F32, tag="rt")
                    nc.vector.reciprocal(rt[:tn], pso[:tn, 64:65])
                    nc.vector.tensor_scalar_mul(
                        o_tm[:tn, it, hh * 64:(hh + 1) * 64],
                        pso[:tn, 0:64], rt[:tn])

        # -------- stage D: O token-major -> feature-major --------
        o_fm = big.tile([128, CC, TOK1], F32R, tag="x_lnT")  # reuses slot
        with tc.tile_pool(name="psD", bufs=4, space="PSUM") as psD:
            for it, (t0, tn) in enumerate(toks):
                for cc in range(CC):
                    pt = psD.tile([128, 128], F32)
                    nc.tensor.transpose(pt[:, :tn],
                                        o_tm[:tn, it, cc * 128:(cc + 1) * 128],
                                        ident_sb[:tn, :tn])
                    nc.vector.tensor_copy(o_fm[:, cc, t0:t0 + tn], pt[:, :tn])

        # -------- stage E: proj --------
        p_fm = big.tile([128, CC, TOK1], F32R, tag="q")  # reuses q slot
        with tc.tile_pool(name="psE", bufs=4, space="PSUM") as psE:
            for cc in range(CC):
                w_sb = load_w1024(wp, cc)
                for (t0, tlen) in ((0, 392), (392, 392)):
                    ps = psE.tile([128, 392], F32)
                    for kc in range(CC):
                        nc.tensor.matmul(ps, w_sb[:, kc],
                                         o_fm[:, kc, t0:t0 + tlen],
                                         start=(kc == 0), stop=(kc == CC - 1))
                    nc.scalar.activation(p_fm[:, cc, t0:t0 + tlen], ps,
                                         AF.Identity, bias=bp_sb[:, cc:cc + 1])

        # -------- stage F: t_fc + residual --------
        xt_fm = big.tile([128, CC, TOK1], F32, tag="k")  # reuses k slot
        with tc.tile_pool(name="psF", bufs=4, space="PSUM") as psF:
            for cc in range(CC):
                w_sb = load_w1024(wfc, cc)
                for (t0, tlen) in ((0, 392), (392, 392)):
                    ps = psF.tile([128, 392], F32)
                    for kc in range(CC):
                        nc.tensor.matmul(ps, w_sb[:, kc],
                                         p_fm[:, kc, t0:t0 + tlen],
                                         start=(kc == 0), stop=(kc == CC - 1))
                    nc.vector.scalar_tensor_tensor(
                        xt_fm[:, cc, t0:t0 + tlen], ps, fcb_sb[:, cc:cc + 1],
                        x_fm_sb[:, cc, t0:t0 + tlen], op0=OP.add, op1=OP.add)
                    nc.sync.dma_start(
                        xt_out.rearrange("(cc p) t -> p cc t", p=128)
                        [:, cc, t0:t0 + tlen], xt_fm[:, cc, t0:t0 + tlen])

    nc.compile()
    return nc


def build_launch2(use_collective=True):
    nc = bacc.Bacc("TRN2", target_bir_lowering=False, debug=False,
                   enable_asserts=False, num_devices=N_CORES)

    def din(name, shape, dt=F32R):
        return nc.dram_tensor(name, shape, dt, kind="ExternalInput").ap()

    xs_tm = din("xs_tm", [TOK2, C], F32)
    res_in = din("res_fm", [C, TOK2], F32)
    wq = din("swq_T", [CC, 128, C]); wk = din("swk_T", [CC, 128, C])
    wv = din("swv_T", [128, CC * V66])
    wp = din("swp_T", [CC, 128, C])
    w1 = din("fc1_T", [HC, 128, C]); w2 = din("fc2_T", [HID, C], BF16)
    # cblock: [0:128 ident][128:1184 vb][1184.. g1,b1,g2,b2,qb,bp,fc2b (8 each)]
    #         [1240:1272 fc1b]
    cblock = din("cblock", [128, 1272], F32)
    out_fm = nc.dram_tensor("out_fm", [C, TOK2], F32, kind="ExternalOutput").ap()
    cls_bnc = nc.dram_tensor("cls_bnc", [128, CC], F32, kind="Internal").ap()
    clsm_bnc = nc.dram_tensor("clsm_bnc", [128, CC], F32, kind="Internal").ap()

    toks2 = _tiles(TOK2)   # 7 tiles: 6x128 + 17

    # token-major index i (0..784) -> spatial flat col (cls dup'd per frame):
    #   i==0 -> cls (cols 197*f); i>=1: f=(i-1)//196, n=(i-1)%196 -> 197f+1+n
    def fm_runs(t0, tn):
        runs = []
        i = max(t0, 1)
        while i < t0 + tn:
            f = (i - 1) // 196
            end = min(t0 + tn, 1 + (f + 1) * 196)
            runs.append((197 * f + 1 + ((i - 1) % 196), end - i, i - t0))
            i = end
        return runs

    with tile.TileContext(nc) as tc, ExitStack() as ctx:
        const = ctx.enter_context(tc.tile_pool(name="const", bufs=1))
        cb = const.tile([128, 1272], F32)
        nc.sync.dma_start(cb, cblock)
        ident_sb = cb[:, 0:128]
        vb_sb = cb[:, 128:128 + V66]
        g1_sb = cb[:, 1184:1192]; b1_sb = cb[:, 1192:1200]
        g2_sb = cb[:, 1200:1208]; b2_sb = cb[:, 1208:1216]
        qb_sb = cb[:, 1216:1224]; bp_sb = cb[:, 1224:1232]
        fc2b_sb = cb[:, 1232:1240]
        fc1b_sb = cb[:, 1240:1272]
        eps_sb = const.tile([128, 1], F32); nc.vector.memset(eps_sb, EPS)
        cls_m = const.tile([128, CC], F32, tag="cls_m")
        cls_part = const.tile([128, CC], F32, tag="cls_part")

        big = ctx.enter_context(tc.tile_pool(name="big", bufs=1))
        wpool = ctx.enter_context(tc.tile_pool(name="w", bufs=3))
        res_sb = big.tile([128, CC, TOK2], F32, tag="hbuf")
        nc.sync.dma_start(res_sb, res_in.rearrange("(cc p) t -> p cc t", p=128))

        def load_w1024(wT, cc):
            w_sb = wpool.tile([128, CC, 128], F32R, tag="w1024")
            nc.sync.dma_start(w_sb, wT[cc].rearrange("p (kc j) -> p kc j", j=128))
            return w_sb

        # -------- stage A: LN1 -> x_lnT (flat 4x197, cls duplicated) --------
        x_lnT = big.tile([128, CC, FLAT2], F32R, tag="x_lnT")
        with tc.tile_pool(name="ln_tmp", bufs=2) as tmp, \
             tc.tile_pool(name="psA", bufs=4, space="PSUM") as psT:
            for (t0, tn) in toks2:
                xt_t = tmp.tile([128, C], F32, tag="xt")
                nc.sync.dma_start(xt_t[:tn], xs_tm[t0:t0 + tn])
                runs = fm_runs(t0, tn)
                if t0 == 0:  # cls goes to col 197*f of every frame
                    runs = runs + [(197 * f, 1, 0) for f in range(FPC)]
                _ln_to_fm(nc, xt_t, tn, x_lnT, runs,
                          g1_sb, b1_sb, eps_sb, psT, tmp, ident_sb)

        # -------- stage B: QKV --------
        # q_fm padded to 256 per frame so logits matmuls get N=256 (fp32r fast)
        q_fm = big.tile([128, CC, FPC, 256], F32R, tag="q")
        for cc in range(CC):
            for f in range(FPC):
                nc.vector.memset(q_fm.bitcast(F32)[:, cc, f, 197:256], 0.0)
        k_fm = big.tile([128, CC, FLAT2], F32R, tag="k")
        halves = ((0, 394), (394, 394))  # frame-boundary-aligned halves of 788
        with tc.tile_pool(name="psB", bufs=4, space="PSUM") as psQ:
            for cc in range(CC):
                w_sb = load_w1024(wq, cc)
                for hi, (t0, tlen) in enumerate(halves):
                    ps = psQ.tile([128, 394], F32)
                    for kc in range(CC):
                        nc.tensor.matmul(ps, w_sb[:, kc],
                                         x_lnT[:, kc, t0:t0 + tlen],
                                         start=(kc == 0), stop=(kc == CC - 1))
                    for fo in range(2):  # two frames per half
                        f = 2 * hi + fo
                        nc.scalar.activation(q_fm[:, cc, f, 0:197],
                                             ps[:, 197 * fo:197 * fo + 197],
                                             AF.Identity, bias=qb_sb[:, cc:cc + 1])
            for cc in range(CC):
                w_sb = load_w1024(wk, cc)
                for (t0, tlen) in halves:
                    ps = psQ.tile([128, 394], F32)
                    for kc in range(CC):
                        nc.tensor.matmul(ps, w_sb[:, kc],
                                         x_lnT[:, kc, t0:t0 + tlen],
                                         start=(kc == 0), stop=(kc == CC - 1))
                    nc.vector.tensor_copy(k_fm[:, cc, t0:t0 + tlen], ps)

        wv_sb = big.tile([128, CC, V66], F32R, tag="wv")
        nc.sync.dma_start(wv_sb, wv.rearrange("p (kc j) -> p kc j", j=V66))
        v_tm = big.tile([128, FPC, 2, V66], F32R, tag="v")
        with tc.tile_pool(name="psV", bufs=4, space="PSUM") as psV:
            for f in range(FPC):
                for ki, (k0, kn) in enumerate(((0, 128), (128, 69))):
                    for (n0, nlen) in ((0, 512), (512, 512), (1024, 32)):
                        ps = psV.tile([128, 512], F32)
                        for kc in range(CC):
                            nc.tensor.matmul(
                                ps[:kn, :nlen],
                                x_lnT[:, kc, 197 * f + k0:197 * f + k0 + kn],
                                wv_sb[:, kc, n0:n0 + nlen],
                                start=(kc == 0), stop=(kc == CC - 1))
                        nc.vector.tensor_add(v_tm[:kn, f, ki, n0:n0 + nlen],
                                             ps[:kn, :nlen],
                                             vb_sb[:kn, n0:n0 + nlen])

        # -------- stage C: spatial attention (197 keys, full) --------
        o_tm = big.tile([128, FPC, 2, C], F32, tag="wv")  # reuses wv slot
        with tc.tile_pool(name="att", bufs=4) as att, \
             tc.tile_pool(name="psL", bufs=3, space="PSUM") as psL, \
             tc.tile_pool(name="psO", bufs=3, space="PSUM") as psO:
            for f in range(FPC):
                for hh in range(H):
                    ch = hh // 2; po = 64 * (hh % 2)
                    a_t = []
                    for ki, (k0, kn) in enumerate(((0, 128), (128, 69))):
                        psl = psL.tile([128, 256], F32)
                        nc.tensor.matmul(
                            psl[:kn, :],
                            k_fm[po:po + 64, ch, 197 * f + k0:197 * f + k0 + kn],
                            q_fm[po:po + 64, ch, f, 0:256],
                            start=True, stop=True)
                        at = att.tile([128, 197], F32R, tag=f"at{ki}")
                        nc.scalar.activation(at[:kn, :], psl[:kn, 0:197], AF.Exp)
                        a_t.append(at)
                    for qi, (q0, qlen) in enumerate(((0, 128), (128, 69))):
                        pso = psO.tile([128, 66], F32)
                        nc.tensor.matmul(pso[:qlen], a_t[0][:, q0:q0 + qlen],
                                         v_tm[:, f, 0, hh * VW:hh * VW + 66],
                                         start=True, stop=False)
                        nc.tensor.matmul(pso[:qlen], a_t[1][:69, q0:q0 + qlen],
                                         v_tm[:69, f, 1, hh * VW:hh * VW + 66],
                                         start=False, stop=True)
                        rt = att.tile([128, 1], F32, tag="rt")
                        nc.vector.reciprocal(rt[:qlen], pso[:qlen, 64:65])
                        nc.vector.tensor_scalar_mul(
                            o_tm[:qlen, f, qi, hh * 64:(hh + 1) * 64],
                            pso[:qlen, 0:64], rt[:qlen])

        # -------- stage D: O -> feature-major --------
        o_fm = big.tile([128, CC, FLAT2], F32R, tag="x_lnT")  # reuses slot
        with tc.tile_pool(name="psD", bufs=4, space="PSUM") as psD:
            for f in range(FPC):
                for qi, (q0, qlen) in enumerate(((0, 128), (128, 69))):
                    for cc in range(CC):
                        pt = psD.tile([128, 128], F32)
                        nc.tensor.transpose(pt[:, :qlen],
                                            o_tm[:qlen, f, qi,
                                                 cc * 128:(cc + 1) * 128],
                                            ident_sb[:qlen, :qlen])
                        nc.vector.tensor_copy(
                            o_fm[:, cc, 197 * f + q0:197 * f + q0 + qlen],
                            pt[:, :qlen])

        # -------- stage E: proj -> p_fm (f32) --------
        p_fm = big.tile([128, CC, FLAT2], F32, tag="q")  # reuses q slot
        with tc.tile_pool(name="psE", bufs=4, space="PSUM") as psE:
            for cc in range(CC):
                w_sb = load_w1024(wp, cc)
                for (t0, tlen) in halves:
                    ps = psE.tile([128, 394], F32)
                    for kc in range(CC):
                        nc.tensor.matmul(ps, w_sb[:, kc],
                                         o_fm[:, kc, t0:t0 + tlen],
                                         start=(kc == 0), stop=(kc == CC - 1))
                    nc.scalar.activation(p_fm[:, cc, t0:t0 + tlen], ps,
                                         AF.Identity, bias=bp_sb[:, cc:cc + 1])

        # -------- stage F: cls partial mean + AllReduce over the pair -------
        cls_view = bass.AP(tensor=p_fm.tensor, offset=p_fm.offset,
                           ap=[list(p_fm.ap[0]), [FLAT2, CC], [197, FPC]])
        nc.vector.tensor_reduce(cls_part, cls_view, axis=mybir.AxisListType.X,
                                op=OP.add)
        nc.scalar.mul(cls_part, cls_part, 1.0 / T)
        nc.sync.dma_start(cls_bnc, cls_part)
        if use_collective:
            nc.gpsimd.collective_compute(
                "AllReduce", OP.add,
                replica_groups=[[0, 1], [2, 3], [4, 5], [6, 7]],
                ins=[cls_bnc], outs=[clsm_bnc])
        else:  # cost-model sim: collective unsupported; timing-equivalent copy
            nc.sync.dma_start(clsm_bnc, cls_bnc)
        nc.sync.dma_start(cls_m, clsm_bnc)

        # -------- stage G: x2 = res + [cls_m | xs] --------
        x2_fm = big.tile([128, CC, TOK2], F32, tag="k")  # reuses k slot
        for cc in range(CC):
            nc.vector.tensor_scalar_add(x2_fm[:, cc, 0:1], res_sb[:, cc, 0:1],
                                        cls_m[:, cc:cc + 1])
            for f in range(FPC):
                nc.vector.tensor_add(
                    x2_fm[:, cc, 1 + 196 * f:1 + 196 * (f + 1)],
                    res_sb[:, cc, 1 + 196 * f:1 + 196 * (f + 1)],
                    p_fm[:, cc, 197 * f + 1:197 * f + 197])

        # -------- stage H: LN2 (fm -> tm -> fm) --------
        x2_lnT = big.tile([128, CC, TOK2 + 1], F32R, tag="v")  # reuses v slot
        for cc in range(CC):
            nc.vector.memset(x2_lnT.bitcast(F32)[:, cc, TOK2:TOK2 + 1], 0.0)
        with tc.tile_pool(name="ln2_tmp", bufs=2) as tmp, \
             tc.tile_pool(name="psH", bufs=6, space="PSUM") as psT:
            for (t0, tn) in toks2:
                x2_t = tmp.tile([128, C], F32, tag="xt")
                for cc in range(CC):
                    pt = psT.tile([128, 128], F32)
                    nc.tensor.transpose(pt[:tn, :], x2_fm[:, cc, t0:t0 + tn],
                                        ident_sb)
                    eng = nc.vector.tensor_copy if cc % 2 else nc.scalar.copy
                    eng(x2_t[:tn, cc * 128:(cc + 1) * 128], pt[:tn, :])
                _ln_to_fm(nc, x2_t, tn, x2_lnT, [(t0, tn, 0)],
                          g2_sb, b2_sb, eps_sb, psT, tmp, ident_sb)

        # -------- stage I: MLP + residual --------
        mhalves = ((0, 394, 394), (394, 392, 391))  # (t0, mm_N, real)
        hbuf = big.tile([128, HC, 394], BF16, tag="hbuf")
        with tc.tile_pool(name="out_t", bufs=3) as outp:
            for (t0, tlen, treal) in mhalves:
                with tc.tile_pool(name=f"psI{t0}", bufs=4, space="PSUM") as psM:
                    for cc in range(HC):
                        w_sb = load_w1024(w1, cc)
                        ps = psM.tile([128, 394], F32)
                        for kc in range(CC):
                            nc.tensor.matmul(ps[:, :tlen], w_sb[:, kc],
                                             x2_lnT[:, kc, t0:t0 + tlen],
                                             start=(kc == 0), stop=(kc == CC - 1))
                        nc.scalar.activation(hbuf[:, cc, :tlen], ps[:, :tlen],
                                             AF.Gelu, bias=fc1b_sb[:, cc:cc + 1])
                with tc.tile_pool(name=f"psI2{t0}", bufs=1, space="PSUM") as psM2, \
                     tc.tile_pool(name=f"w2p{t0}", bufs=3) as w2pool:
                    pss = [psM2.tile([128, 394], F32, tag=f"o{cc}",
                                     name=f"ps_fc2_{t0}_{cc}")
                           for cc in range(CC)]
                    for kc in range(HC):
                        w_sb = w2pool.tile([128, C], BF16, tag="w2row")
                        nc.sync.dma_start(w_sb, w2[kc * 128:(kc + 1) * 128, :])
                        for cc in range(CC):
                            nc.tensor.matmul(pss[cc][:, :tlen],
                                             w_sb[:, cc * 128:(cc + 1) * 128],
                                             hbuf[:, kc, :tlen],
                                             start=(kc == 0), stop=(kc == HC - 1))
                    for cc in range(CC):
                        ot = outp.tile([128, 394], F32, tag="ot")
                        nc.vector.scalar_tensor_tensor(
                            ot[:, :treal], pss[cc][:, :treal],
                            fc2b_sb[:, cc:cc + 1], x2_fm[:, cc, t0:t0 + treal],
                            op0=OP.add, op1=OP.add)
                        nc.sync.dma_start(
                            out_fm.rearrange("(cc p) t -> p cc t", p=128)
                            [:, cc, t0:t0 + treal], ot[:, :treal])

    nc.compile()
    return nc


# ---------------------------------------------------------------------------
# host side
# ---------------------------------------------------------------------------
_NC1 = None
_NC2 = None


def _get_programs():
    global _NC1, _NC2
    if _NC1 is None:
        _NC1 = build_launch1()
        _NC2 = build_launch2()
    return _NC1, _NC2


def _pre(wT):
    """(Cin, Cout) -> (Cout//128, 128, Cin): [cc, p, kc*128+j] = wT[kc*128+p, cc*128+j]
    so each cc-chunk DMA is fully contiguous."""
    cin, cout = wT.shape
    arr = wT.reshape(cin // 128, 128, cout // 128, 128)
    return np.ascontiguousarray(arr.transpose(2, 1, 0, 3).reshape(
        cout // 128, 128, cin))


def _pre_v(wva):
    """(C, V66) -> (128, CC*V66): [p, kc*V66 + j] = wva[kc*128+p, j]."""
    arr = wva.reshape(CC, 128, V66)
    return np.ascontiguousarray(arr.transpose(1, 0, 2).reshape(128, CC * V66))


def _aug_v(wv_t, vb):
    """wv_t: (C, C) = wv.T -> (C, 1040) with ones-slot col per head zeroed;
    vb -> (1040,) with 1.0 in the ones-slot."""
    wva = np.zeros((C, V66), np.float32)
    vba = np.zeros((V66,), np.float32)
    for hh in range(H):
        wva[:, hh * VW:hh * VW + 64] = wv_t[:, hh * 64:(hh + 1) * 64]
        vba[hh * VW:hh * VW + 64] = vb[hh * 64:(hh + 1) * 64]
        vba[hh * VW + 64] = 1.0
    return wva, vba


def kernel(x, cls_token, tln_g, tln_b, t_wqkv, t_qb, t_vb, t_wproj, t_bproj,
           t_fc_w, t_fc_b, n1_g, n1_b, s_wqkv, s_qb, s_vb, s_wproj, s_bproj,
           n2_g, n2_b, fc1_w, fc1_b, fc2_w, fc2_b):
    x = np.asarray(x, np.float32)
    nc1, nc2 = _get_programs()
    f32 = np.float32
    cont = np.ascontiguousarray

    ident = np.eye(128, dtype=f32)
    mask = np.zeros((128, 128), f32)
    for g in range(16):
        mask[g * 8:(g + 1) * 8, g * 8:(g + 1) * 8] = 1.0

    def cols(v):  # (k*128,) -> (128, k)
        v = np.asarray(v, f32)
        return v.reshape(-1, 128).T

    # launch 1 prep
    twq_T = _pre((np.asarray(t_wqkv)[0:C] * SCALE).T.astype(f32))
    twk_T = _pre(np.asarray(t_wqkv)[C:2 * C].T.astype(f32))
    twv_T = cont(np.asarray(t_wqkv)[2 * C:3 * C].T.astype(f32))
    twv_aug, tvb_aug = _aug_v(twv_T, np.asarray(t_vb, f32))
    twv_aug = _pre_v(twv_aug)
    twp_T = _pre(np.asarray(t_wproj).T.astype(f32))
    tfc_T = _pre(np.asarray(t_fc_w).T.astype(f32))
    tqb_s = (np.asarray(t_qb, f32) * SCALE).astype(f32)
    cb1 = np.concatenate([
        ident, mask, np.broadcast_to(tvb_aug, (128, V66)),
        cols(tln_g), cols(tln_b), cols(tqb_s), cols(t_bproj), cols(t_fc_b),
    ], axis=1).astype(f32)
    cb1 = np.ascontiguousarray(cb1)

    # x in (n, t) token order per (b, h): x[b, t, h*98+nl, :] at row nl*8+t
    xp = np.transpose(x, (0, 2, 1, 3))          # (B, N, T, C)
    in_maps1 = []
    for core in range(N_CORES):
        b, h = core // 2, core % 2
        xtm = cont(xp[b, h * NH:(h + 1) * NH].reshape(TOK1, C))
        in_maps1.append({
            "x_tm": xtm, "x_fm": cont(xtm.T),
            "wq_T": twq_T, "wk_T": twk_T, "wv_T": twv_aug,
            "wp_T": twp_T, "wfc_T": tfc_T, "cblock": cb1,
        })
    res1 = run_bass_kernel_spmd(nc1, in_maps1, core_ids=list(range(N_CORES)))

    # xt_global[b, n*8+t, :] from per-core feature-major outputs
    xt_g = np.empty((B, N * T, C), f32)
    for core in range(N_CORES):
        b, h = core // 2, core % 2
        arr = res1.results[core]["xt_out"]      # (C, 784)
        xt_g[b, h * NH * T:(h + 1) * NH * T] = arr.T

    # launch 2 prep
    swq_T = _pre((np.asarray(s_wqkv)[0:C] * SCALE).T.astype(f32))
    swk_T = _pre(np.asarray(s_wqkv)[C:2 * C].T.astype(f32))
    swv_T = cont(np.asarray(s_wqkv)[2 * C:3 * C].T.astype(f32))
    swv_aug, svb_aug = _aug_v(swv_T, np.asarray(s_vb, f32))
    swv_aug = _pre_v(swv_aug)
    swp_T = _pre(np.asarray(s_wproj).T.astype(f32))
    fc1_T = _pre(np.asarray(fc1_w).T.astype(f32))
    fc2_T = cont(np.asarray(fc2_w).T.astype(ml_dtypes.bfloat16))
    sqb_s = (np.asarray(s_qb, f32) * SCALE).astype(f32)
    cls_np = np.asarray(cls_token, f32)
    cb2 = np.concatenate([
        ident, np.broadcast_to(svb_aug, (128, V66)),
        cols(n1_g), cols(n1_b), cols(n2_g), cols(n2_b), cols(sqb_s),
        cols(s_bproj), cols(fc2_b), cols(fc1_b),
    ], axis=1).astype(f32)
    cb2 = np.ascontiguousarray(cb2)

    in_maps2 = []
    for core in range(N_CORES):
        b, h = core // 2, core % 2
        # rows: [cls_b] + xt[b, n*8 + (4h+fl)] for fl, n  (frame-major)
        xt_b = xt_g[b].reshape(N, T, C)
        rows = xt_b[:, 4 * h:4 * h + 4].transpose(1, 0, 2).reshape(FPC * N, C)
        xstm = np.concatenate([cls_np[b:b + 1], rows], axis=0)
        in_maps2.append({
            "xs_tm": cont(xstm), "res_fm": cont(xstm.T),
            "swq_T": swq_T, "swk_T": swk_T, "swv_T": swv_aug, "swp_T": swp_T,
            "fc1_T": fc1_T, "fc2_T": fc2_T, "cblock": cb2,
        })
    res2 = run_bass_kernel_spmd(nc2, in_maps2, core_ids=list(range(N_CORES)))

    xo = np.empty((B, T, N, C), f32)
    cls_out = np.empty((B, C), f32)
    for core in range(N_CORES):
        b, h = core // 2, core % 2
        arr = res2.results[core]["out_fm"]      # (C, 785)
        body = arr[:, 1:].reshape(C, FPC, N)
        for fl in range(FPC):
            xo[b, 4 * h + fl] = body[:, fl, :].T
        if h == 0:
            cls_out[b] = arr[:, 0]
    return xo, cls_out


# revision 19
# speedup vs baseline: 20684.9072x; 20684.9072x over previous
"""TimeSformer-style divided space-time attention block on 8 trn2 NeuronCores.

Sharding: core = 2*b + h (b in 0..3, h in 0..1).
  Launch 1 (temporal attn + fc + residual): core handles batch b, patch-half h
    (98 of 196 patches), all T=8 frames. 784 tokens/core in (n_loc, t) order.
  Launch 2 (spatial attn + cls + MLP):      core handles batch b, frame-half h
    (4 of 8 frames), all 196 patches + cls. Host reshuffles between launches.

Layouts: activations feature-major (C on partitions) for matmuls; LN/softmax
stats token-major via PE transposes; weights host-pre-transposed; q-scale
folded into wq/qb; softmax denominator via ones-column appended to V.
Matmuls in float32r (except fc2: bf16), everything else fp32, exact erf-GELU.
"""

import numpy as np
import ml_dtypes
from contextlib import ExitStack

import concourse.bass as bass
import concourse.tile as tile
from concourse import bacc, mybir
from concourse.bass_utils import run_bass_kernel_spmd

F32 = mybir.dt.float32
F32R = mybir.dt.float32r
BF16 = mybir.dt.bfloat16
AF = mybir.ActivationFunctionType
OP = mybir.AluOpType

B, T, N, C = 4, 8, 196, 1024
H, D = 16, 64
SCALE = D ** -0.5
HID = 4 * C
EPS = 1e-5
NH = N // 2            # 98 patches per core in launch 1
TOK1 = NH * T          # 784 tokens per core, launch 1
FPC = T // 2           # 4 frames per core in launch 2
TOK2 = 1 + FPC * N     # 785 tokens per core, launch 2 (cls + 4*196)
FLAT2 = FPC * 197      # 788: spatial qkv token list, cls duplicated per frame
CC = C // 128          # 8 chunks of C
HC = HID // 128        # 32 chunks of HID
VW = 66                # per-head V block: 64 V + ones col + pad (even N)
V66 = H * VW           # 1056
N_CORES = 8


def _tiles(total, step=128):
    return [(i, min(step, total - i)) for i in range(0, total, step)]


def _bcast_row(ap_1d, parts=128):
    """DRAM (n,) -> DMA-source AP broadcasting over `parts` partitions."""
    return bass.AP(tensor=ap_1d.tensor, offset=ap_1d.offset,
                   ap=[[0, parts]] + list(ap_1d.ap))


def _ln_to_fm(nc, tm_src_tile, tn, dst_fm, dst_cols, g_sb, b_sb,
              eps_sb, psT, tmp, ident_sb):
    """LN a token-major tile (tn<=128 tokens x C) and write normalized*g+b
    transposed into dst_fm[:, cc, ...] (feature-major, fp32r).
    dst_cols: list of (col0, length, src0) runs."""
    st = tmp.tile([128, 2, 6], F32, tag="st")
    nc.vector.bn_stats(st[:tn, 0], tm_src_tile[:tn, 0:512])
    nc.vector.bn_stats(st[:tn, 1], tm_src_tile[:tn, 512:1024])
    mv = tmp.tile([128, 2], F32, tag="mv")
    nc.vector.bn_aggr(mv[:tn], st[:tn])
    rstd = tmp.tile([128, 1], F32, tag="rstd")
    nc.scalar.activation(rstd[:tn], mv[:tn, 1:2], AF.Sqrt, bias=eps_sb[:tn])
    nc.vector.reciprocal(rstd[:tn], rstd[:tn])
    y = tmp.tile([128, C], F32, tag="y")
    nc.vector.tensor_scalar(y[:tn], tm_src_tile[:tn], mv[:tn, 0:1], rstd[:tn],
                            op0=OP.subtract, op1=OP.mult)
    for cc in range(CC):
        pt = psT.tile([128, 128], F32)
        nc.tensor.transpose(pt[:, :tn], y[:tn, cc * 128:(cc + 1) * 128],
                            ident_sb[:tn, :tn])
        for (col0, length, src0) in dst_cols:
            nc.scalar.activation(dst_fm[:, cc, col0:col0 + length],
                                 pt[:, src0:src0 + length], AF.Identity,
                                 bias=b_sb[:, cc:cc + 1], scale=g_sb[:, cc:cc + 1])


def build_launch1():
    nc = bacc.Bacc("TRN2", target_bir_lowering=False, debug=False,
                   enable_asserts=False, num_devices=N_CORES)

    def din(name, shape, dt=F32R):
        return nc.dram_tensor(name, shape, dt, kind="ExternalInput").ap()

    x_tm = din("x_tm", [TOK1, C], F32)
    x_fm = din("x_fm", [C, TOK1], F32)
    wq = din("wq_T", [CC, 128, C]); wk = din("wk_T", [CC, 128, C])
    wv = din("wv_T", [128, CC * V66])
    wp = din("wp_T", [CC, 128, C]); wfc = din("wfc_T", [CC, 128, C])
    # cblock: [0:128 ident][128:256 mask][256:1312 vb_bcast]
    #         [1312.. g,b,qb,bp,fcb cols (8 each)]
    cblock = din("cblock", [128, 1352], F32)
    xt_out = nc.dram_tensor("xt_out", [C, TOK1], F32, kind="ExternalOutput").ap()

    toks = _tiles(TOK1)  # 7 tiles: 6x128 + 16

    with tile.TileContext(nc) as tc, ExitStack() as ctx:
        const = ctx.enter_context(tc.tile_pool(name="const", bufs=1))
        cb = const.tile([128, 1352], F32)
        nc.sync.dma_start(cb, cblock)
        ident_sb = cb[:, 0:128]; mask_sb = cb[:, 128:256]
        vb_sb = cb[:, 256:256 + V66]
        g_sb = cb[:, 1312:1320]; b_sb = cb[:, 1320:1328]
        qb_sb = cb[:, 1328:1336]; bp_sb = cb[:, 1336:1344]
        fcb_sb = cb[:, 1344:1352]
        eps_sb = const.tile([128, 1], F32); nc.vector.memset(eps_sb, EPS)

        big = ctx.enter_context(tc.tile_pool(name="big", bufs=1))
        wpool = ctx.enter_context(tc.tile_pool(name="w", bufs=3))
        def load_w1024(wT, cc):
            w_sb = wpool.tile([128, CC, 128], F32R, tag="w1024")
            nc.sync.dma_start(w_sb, wT[cc].rearrange("p (kc j) -> p kc j", j=128))
            return w_sb

        # -------- stage A: LN + transpose -> x_lnT --------
        x_lnT = big.tile([128, CC, TOK1], F32R, tag="x_lnT")
        with tc.tile_pool(name="ln_tmp", bufs=2) as tmp, \
             tc.tile_pool(name="psA", bufs=4, space="PSUM") as psT:
            for (t0, tn) in toks:
                xt_t = tmp.tile([128, C], F32, tag="xt")
                nc.sync.dma_start(xt_t[:tn], x_tm[t0:t0 + tn])
                _ln_to_fm(nc, xt_t, tn, x_lnT, [(t0, tn, 0)],
                          g_sb, b_sb, eps_sb, psT, tmp, ident_sb)


        # -------- stage B: QKV --------
        q_fm = big.tile([128, CC, TOK1], F32R, tag="q")
        k_fm = big.tile([128, CC, TOK1], F32R, tag="k")
        with tc.tile_pool(name="psB", bufs=4, space="PSUM") as psQ:
            for (dst, wT, bias_cols) in ((q_fm, wq, qb_sb), (k_fm, wk, None)):
                for cc in range(CC):
                    w_sb = load_w1024(wT, cc)
                    for (t0, tlen) in ((0, 392), (392, 392)):
                        ps = psQ.tile([128, 392], F32)
                        for kc in range(CC):
                            nc.tensor.matmul(ps, w_sb[:, kc],
                                             x_lnT[:, kc, t0:t0 + tlen],
                                             start=(kc == 0), stop=(kc == CC - 1))
                        if bias_cols is None:
                            nc.vector.tensor_copy(dst[:, cc, t0:t0 + tlen], ps)
                        else:
                            nc.scalar.activation(dst[:, cc, t0:t0 + tlen], ps,
                                                 AF.Identity,
                                                 bias=bias_cols[:, cc:cc + 1])

        wv_sb = big.tile([128, CC, V66], F32R, tag="wv")
        nc.sync.dma_start(wv_sb, wv.rearrange("p (kc j) -> p kc j", j=V66))
        v_tm = big.tile([128, len(toks), V66], F32R, tag="v")
        with tc.tile_pool(name="psV", bufs=4, space="PSUM") as psV:
            for it, (t0, tn) in enumerate(toks):
                for (n0, nlen) in ((0, 512), (512, 512), (1024, 32)):
                    ps = psV.tile([128, 512], F32)
                    for kc in range(CC):
                        nc.tensor.matmul(ps[:tn, :nlen],
                                         x_lnT[:, kc, t0:t0 + tn],
                                         wv_sb[:, kc, n0:n0 + nlen],
                                         start=(kc == 0), stop=(kc == CC - 1))
                    nc.vector.tensor_add(v_tm[:tn, it, n0:n0 + nlen],
                                         ps[:tn, :nlen], vb_sb[:tn, n0:n0 + nlen])

        # -------- stage C: temporal attention (block-diag-8 within tile) -----
        o_tm = big.tile([128, len(toks), C], F32, tag="wv")  # reuses wv slot
        with tc.tile_pool(name="att", bufs=6) as att, \
             tc.tile_pool(name="psL", bufs=3, space="PSUM") as psL, \
             tc.tile_pool(name="psO", bufs=3, space="PSUM") as psO:
            for it, (t0, tn) in enumerate(toks):
                qn = min(256, TOK1 - t0)
                for hh in range(H):
                    ch = hh // 2; po = 64 * (hh % 2)
                    psl = psL.tile([128, 256], F32)
                    nc.tensor.matmul(psl[:tn, :qn],
                                     k_fm[po:po + 64, ch, t0:t0 + tn],
                                     q_fm[po:po + 64, ch, t0:t0 + qn],
                                     start=True, stop=True)
                    et = att.tile([128, 128], F32, tag="et")
                    nc.scalar.activation(et[:tn, :tn], psl[:tn, 0:tn], AF.Exp)
                    at = att.tile([128, 128], F32R, tag="at")
                    nc.vector.tensor_mul(at[:tn, :tn], et[:tn, :tn],
                                         mask_sb[:tn, :tn])
                    pso = psO.tile([128, 66], F32)
                    nc.tensor.matmul(pso[:tn], at[:tn, :tn],
                                     v_tm[:tn, it, hh * VW:hh * VW + 66],
                                     start=True, stop=True)
                    rt = att.tile([128, 1], F32, tag="rt")
                    nc.vector.reciprocal(rt[:tn], pso[:tn, 64:65])
                    nc.vector.tensor_scalar_mul(
                        o_tm[:tn, it, hh * 64:(hh + 1) * 64],
                        pso[:tn, 0:64], rt[:tn])

        # -------- stage D: O token-major -> feature-major --------
        o_fm = big.tile([128, CC, TOK1], F32R, tag="x_lnT")  # reuses slot
        with tc.tile_pool(name="psD", bufs=4, space="PSUM") as psD:
            for it, (t0, tn) in enumerate(toks):
                for cc in range(CC):
                    pt = psD.tile([128, 128], F32)
                    nc.tensor.transpose(pt[:, :tn],
                                        o_tm[:tn, it, cc * 128:(cc + 1) * 128],
                                        ident_sb[:tn, :tn])
                    nc.vector.tensor_copy(o_fm[:, cc, t0:t0 + tn], pt[:, :tn])

        # -------- stage E: proj --------
        p_fm = big.tile([128, CC, TOK1], F32R, tag="q")  # reuses q slot
        with tc.tile_pool(name="psE", bufs=4, space="PSUM") as psE:
            for cc in range(CC):
                w_sb = load_w1024(wp, cc)
                for (t0, tlen) in ((0, 392), (392, 392)):
                    ps = psE.tile([128, 392], F32)
                    for kc in range(CC):
                        nc.tensor.matmul(ps, w_sb[:, kc],
                                         o_fm[:, kc, t0:t0 + tlen],
                                         start=(kc == 0), stop=(kc == CC - 1))
                    nc.scalar.activation(p_fm[:, cc, t0:t0 + tlen], ps,
                                         AF.Identity, bias=bp_sb[:, cc:cc + 1])

        # -------- stage F: t_fc + residual --------
        x_fm_sb = big.tile([128, CC, TOK1], F32, tag="v")  # reuses v slot
        nc.sync.dma_start(x_fm_sb, x_fm.rearrange("(cc p) t -> p cc t", p=128))
        xt_fm = big.tile([128, CC, TOK1], F32, tag="k")  # reuses k slot
        with tc.tile_pool(name="psF", bufs=4, space="PSUM") as psF:
            for cc in range(CC):
                w_sb = load_w1024(wfc, cc)
                for (t0, tlen) in ((0, 392), (392, 392)):
                    ps = psF.tile([128, 392], F32)
                    for kc in range(CC):
                        nc.tensor.matmul(ps, w_sb[:, kc],
                                         p_fm[:, kc, t0:t0 + tlen],
                                         start=(kc == 0), stop=(kc == CC - 1))
                    nc.vector.scalar_tensor_tensor(
                        xt_fm[:, cc, t0:t0 + tlen], ps, fcb_sb[:, cc:cc + 1],
                        x_fm_sb[:, cc, t0:t0 + tlen], op0=OP.add, op1=OP.add)
                    nc.sync.dma_start(
                        xt_out.rearrange("(cc p) t -> p cc t", p=128)
                        [:, cc, t0:t0 + tlen], xt_fm[:, cc, t0:t0 + tlen])

    nc.compile()
    return nc


def build_launch2(use_collective=True):
    nc = bacc.Bacc("TRN2", target_bir_lowering=False, debug=False,
                   enable_asserts=False, num_devices=N_CORES)

    def din(name, shape, dt=F32R):
        return nc.dram_tensor(name, shape, dt, kind="ExternalInput").ap()

    xs_tm = din("xs_tm", [TOK2, C], F32)
    res_in = din("res_fm", [C, TOK2], F32)
    wq = din("swq_T", [CC, 128, C]); wk = din("swk_T", [CC, 128, C])
    wv = din("swv_T", [128, CC * V66])
    wp = din("swp_T", [CC, 128, C])
    w1 = din("fc1_T", [HC, 128, C]); w2 = din("fc2_T", [HID, C], BF16)
    # cblock: [0:128 ident][128:1184 vb][1184.. g1,b1,g2,b2,qb,bp,fc2b (8 each)]
    #         [1240:1272 fc1b]
    cblock = din("cblock", [128, 1272], F32)
    out_fm = nc.dram_tensor("out_fm", [C, TOK2], F32, kind="ExternalOutput").ap()
    cls_bnc = nc.dram_tensor("cls_bnc", [128, CC], F32, kind="Internal").ap()
    clsm_bnc = nc.dram_tensor("clsm_bnc", [128, CC], F32, kind="Internal").ap()

    toks2 = _tiles(TOK2)   # 7 tiles: 6x128 + 17

    # token-major index i (0..784) -> spatial flat col (cls dup'd per frame):
    #   i==0 -> cls (cols 197*f); i>=1: f=(i-1)//196, n=(i-1)%196 -> 197f+1+n
    def fm_runs(t0, tn):
        runs = []
        i = max(t0, 1)
        while i < t0 + tn:
            f = (i - 1) // 196
            end = min(t0 + tn, 1 + (f + 1) * 196)
            runs.append((197 * f + 1 + ((i - 1) % 196), end - i, i - t0))
            i = end
        return runs

    with tile.TileContext(nc) as tc, ExitStack() as ctx:
        const = ctx.enter_context(tc.tile_pool(name="const", bufs=1))
        cb = const.tile([128, 1272], F32)
        nc.sync.dma_start(cb, cblock)
        ident_sb = cb[:, 0:128]
        vb_sb = cb[:, 128:128 + V66]
        g1_sb = cb[:, 1184:1192]; b1_sb = cb[:, 1192:1200]
        g2_sb = cb[:, 1200:1208]; b2_sb = cb[:, 1208:1216]
        qb_sb = cb[:, 1216:1224]; bp_sb = cb[:, 1224:1232]
        fc2b_sb = cb[:, 1232:1240]
        fc1b_sb = cb[:, 1240:1272]
        eps_sb = const.tile([128, 1], F32); nc.vector.memset(eps_sb, EPS)
        cls_m = const.tile([128, CC], F32, tag="cls_m")
        cls_part = const.tile([128, CC], F32, tag="cls_part")

        big = ctx.enter_context(tc.tile_pool(name="big", bufs=1))
        wpool = ctx.enter_context(tc.tile_pool(name="w", bufs=3))
        def load_w1024(wT, cc):
            w_sb = wpool.tile([128, CC, 128], F32R, tag="w1024")
            nc.sync.dma_start(w_sb, wT[cc].rearrange("p (kc j) -> p kc j", j=128))
            return w_sb

        # -------- stage A: LN1 -> x_lnT (flat 4x197, cls duplicated) --------
        x_lnT = big.tile([128, CC, FLAT2], F32R, tag="x_lnT")
        with tc.tile_pool(name="ln_tmp", bufs=2) as tmp, \
             tc.tile_pool(name="psA", bufs=4, space="PSUM") as psT:
            for (t0, tn) in toks2:
                xt_t = tmp.tile([128, C], F32, tag="xt")
                nc.sync.dma_start(xt_t[:tn], xs_tm[t0:t0 + tn])
                runs = fm_runs(t0, tn)
                if t0 == 0:  # cls goes to col 197*f of every frame
                    runs = runs + [(197 * f, 1, 0) for f in range(FPC)]
                _ln_to_fm(nc, xt_t, tn, x_lnT, runs,
                          g1_sb, b1_sb, eps_sb, psT, tmp, ident_sb)
        res_sb = big.tile([128, CC, TOK2], F32, tag="hbuf")
        nc.sync.dma_start(res_sb, res_in.rearrange("(cc p) t -> p cc t", p=128))

        # -------- stage B: QKV --------
        wv_sb = big.tile([128, CC, V66], F32R, tag="wv")
        nc.sync.dma_start(wv_sb, wv.rearrange("p (kc j) -> p kc j", j=V66))
        v_tm = big.tile([128, FPC, 2, V66], F32R, tag="v")
        with tc.tile_pool(name="psV", bufs=4, space="PSUM") as psV:
            for f in range(FPC):
                for ki, (k0, kn) in enumerate(((0, 128), (128, 69))):
                    for (n0, nlen) in ((0, 512), (512, 512), (1024, 32)):
                        ps = psV.tile([128, 512], F32)
                        for kc in range(CC):
                            nc.tensor.matmul(
                                ps[:kn, :nlen],
                                x_lnT[:, kc, 197 * f + k0:197 * f + k0 + kn],
                                wv_sb[:, kc, n0:n0 + nlen],
                                start=(kc == 0), stop=(kc == CC - 1))
                        nc.vector.tensor_add(v_tm[:kn, f, ki, n0:n0 + nlen],
                                             ps[:kn, :nlen],
                                             vb_sb[:kn, n0:n0 + nlen])

        # q_fm padded to 256 per frame so logits matmuls get N=256 (fp32r fast)
        q_fm = big.tile([128, CC, FPC, 256], F32R, tag="q")
        for cc in range(CC):
            for f in range(FPC):
                nc.vector.memset(q_fm.bitcast(F32)[:, cc, f, 197:256], 0.0)
        k_fm = big.tile([128, CC, FLAT2], F32R, tag="k")
        halves = ((0, 394), (394, 394))  # frame-boundary-aligned halves of 788
        with tc.tile_pool(name="psB", bufs=4, space="PSUM") as psQ:
            for cc in range(CC):
                w_sb = load_w1024(wq, cc)
                for hi, (t0, tlen) in enumerate(halves):
                    ps = psQ.tile([128, 394], F32)
                    for kc in range(CC):
                        nc.tensor.matmul(ps, w_sb[:, kc],
                                         x_lnT[:, kc, t0:t0 + tlen],
                                         start=(kc == 0), stop=(kc == CC - 1))
                    for fo in range(2):  # two frames per half
                        f = 2 * hi + fo
                        nc.scalar.activation(q_fm[:, cc, f, 0:197],
                                             ps[:, 197 * fo:197 * fo + 197],
                                             AF.Identity, bias=qb_sb[:, cc:cc + 1])
            for cc in range(CC):
                w_sb = load_w1024(wk, cc)
                for (t0, tlen) in halves:
                    ps = psQ.tile([128, 394], F32)
                    for kc in range(CC):
                        nc.tensor.matmul(ps, w_sb[:, kc],
                                         x_lnT[:, kc, t0:t0 + tlen],
                                         start=(kc == 0), stop=(kc == CC - 1))
                    nc.vector.tensor_copy(k_fm[:, cc, t0:t0 + tlen], ps)

        # -------- stage C: spatial attention (197 keys, full) --------
        o_tm = big.tile([128, FPC, 2, C], F32, tag="wv")  # reuses wv slot
        with tc.tile_pool(name="att", bufs=4) as att, \
             tc.tile_pool(name="psL", bufs=3, space="PSUM") as psL, \
             tc.tile_pool(name="psO", bufs=3, space="PSUM") as psO:
            for f in range(FPC):
                for hh in range(H):
                    ch = hh // 2; po = 64 * (hh % 2)
                    a_t = []
                    for ki, (k0, kn) in enumerate(((0, 128), (128, 69))):
                        psl = psL.tile([128, 256], F32)
                        nc.tensor.matmul(
                            psl[:kn, :],
                            k_fm[po:po + 64, ch, 197 * f + k0:197 * f + k0 + kn],
                            q_fm[po:po + 64, ch, f, 0:256],
                            start=True, stop=True)
                        at = att.tile([128, 197], F32R, tag=f"at{ki}")
                        nc.scalar.activation(at[:kn, :], psl[:kn, 0:197], AF.Exp)
                        a_t.append(at)
                    for qi, (q0, qlen) in enumerate(((0, 128), (128, 69))):
                        pso = psO.tile([128, 66], F32)
                        nc.tensor.matmul(pso[:qlen], a_t[0][:, q0:q0 + qlen],
                                         v_tm[:, f, 0, hh * VW:hh * VW + 66],
                                         start=True, stop=False)
                        nc.tensor.matmul(pso[:qlen], a_t[1][:69, q0:q0 + qlen],
                                         v_tm[:69, f, 1, hh * VW:hh * VW + 66],
                                         start=False, stop=True)
                        rt = att.tile([128, 1], F32, tag="rt")
                        nc.vector.reciprocal(rt[:qlen], pso[:qlen, 64:65])
                        nc.vector.tensor_scalar_mul(
                            o_tm[:qlen, f, qi, hh * 64:(hh + 1) * 64],
                            pso[:qlen, 0:64], rt[:qlen])

        # -------- stage D: O -> feature-major --------
        o_fm = big.tile([128, CC, FLAT2], F32R, tag="x_lnT")  # reuses slot
        with tc.tile_pool(name="psD", bufs=4, space="PSUM") as psD:
            for f in range(FPC):
                for qi, (q0, qlen) in enumerate(((0, 128), (128, 69))):
                    for cc in range(CC):
                        pt = psD.tile([128, 128], F32)
                        nc.tensor.transpose(pt[:, :qlen],
                                            o_tm[:qlen, f, qi,
                                                 cc * 128:(cc + 1) * 128],
                                            ident_sb[:qlen, :qlen])
                        nc.vector.tensor_copy(
                            o_fm[:, cc, 197 * f + q0:197 * f + q0 + qlen],
                            pt[:, :qlen])

        # -------- stage E: proj -> p_fm (f32) --------
        p_fm = big.tile([128, CC, FLAT2], F32, tag="q")  # reuses q slot
        with tc.tile_pool(name="psE", bufs=4, space="PSUM") as psE:
            for cc in range(CC):
                w_sb = load_w1024(wp, cc)
                for (t0, tlen) in halves:
                    ps = psE.tile([128, 394], F32)
                    for kc in range(CC):
                        nc.tensor.matmul(ps, w_sb[:, kc],
                                         o_fm[:, kc, t0:t0 + tlen],
                                         start=(kc == 0), stop=(kc == CC - 1))
                    nc.scalar.activation(p_fm[:, cc, t0:t0 + tlen], ps,
                                         AF.Identity, bias=bp_sb[:, cc:cc + 1])

        # -------- stage F: cls partial mean + AllReduce over the pair -------
        cls_view = bass.AP(tensor=p_fm.tensor, offset=p_fm.offset,
                           ap=[list(p_fm.ap[0]), [FLAT2, CC], [197, FPC]])
        nc.vector.tensor_reduce(cls_part, cls_view, axis=mybir.AxisListType.X,
                                op=OP.add)
        nc.scalar.mul(cls_part, cls_part, 1.0 / T)
        nc.sync.dma_start(cls_bnc, cls_part)
        if use_collective:
            nc.gpsimd.collective_compute(
                "AllReduce", OP.add,
                replica_groups=[[0, 1], [2, 3], [4, 5], [6, 7]],
                ins=[cls_bnc], outs=[clsm_bnc])
        else:  # cost-model sim: collective unsupported; timing-equivalent copy
            nc.sync.dma_start(clsm_bnc, cls_bnc)
        nc.sync.dma_start(cls_m, clsm_bnc)

        # -------- stage G: x2 = res + [cls_m | xs] --------
        x2_fm = big.tile([128, CC, TOK2], F32, tag="k")  # reuses k slot
        for cc in range(CC):
            nc.vector.tensor_scalar_add(x2_fm[:, cc, 0:1], res_sb[:, cc, 0:1],
                                        cls_m[:, cc:cc + 1])
            for f in range(FPC):
                nc.vector.tensor_add(
                    x2_fm[:, cc, 1 + 196 * f:1 + 196 * (f + 1)],
                    res_sb[:, cc, 1 + 196 * f:1 + 196 * (f + 1)],
                    p_fm[:, cc, 197 * f + 1:197 * f + 197])

        # -------- stage H: LN2 (fm -> tm -> fm) --------
        x2_lnT = big.tile([128, CC, TOK2 + 1], F32R, tag="v")  # reuses v slot
        for cc in range(CC):
            nc.vector.memset(x2_lnT.bitcast(F32)[:, cc, TOK2:TOK2 + 1], 0.0)
        with tc.tile_pool(name="ln2_tmp", bufs=2) as tmp, \
             tc.tile_pool(name="psH", bufs=6, space="PSUM") as psT:
            for (t0, tn) in toks2[3:] + toks2[1:3] + toks2[:1]:
                x2_t = tmp.tile([128, C], F32, tag="xt")
                for cc in range(CC):
                    pt = psT.tile([128, 128], F32)
                    nc.tensor.transpose(pt[:tn, :], x2_fm[:, cc, t0:t0 + tn],
                                        ident_sb)
                    eng = nc.vector.tensor_copy if cc % 2 else nc.scalar.copy
                    eng(x2_t[:tn, cc * 128:(cc + 1) * 128], pt[:tn, :])
                _ln_to_fm(nc, x2_t, tn, x2_lnT, [(t0, tn, 0)],
                          g2_sb, b2_sb, eps_sb, psT, tmp, ident_sb)

        # -------- stage I: MLP + residual --------
        mhalves = ((394, 392, 391), (0, 394, 394))  # (t0, mm_N, real)
        hbuf = big.tile([128, HC, 394], BF16, tag="hbuf")
        with tc.tile_pool(name="out_t", bufs=3) as outp:
            for (t0, tlen, treal) in mhalves:
                with tc.tile_pool(name=f"psI{t0}", bufs=4, space="PSUM") as psM:
                    for cc in range(HC):
                        w_sb = load_w1024(w1, cc)
                        ps = psM.tile([128, 394], F32)
                        for kc in range(CC):
                            nc.tensor.matmul(ps[:, :tlen], w_sb[:, kc],
                                             x2_lnT[:, kc, t0:t0 + tlen],
                                             start=(kc == 0), stop=(kc == CC - 1))
                        nc.scalar.activation(hbuf[:, cc, :tlen], ps[:, :tlen],
                                             AF.Gelu, bias=fc1b_sb[:, cc:cc + 1])
                with tc.tile_pool(name=f"psI2{t0}", bufs=1, space="PSUM") as psM2, \
                     tc.tile_pool(name=f"w2p{t0}", bufs=3) as w2pool:
                    pss = [psM2.tile([128, 394], F32, tag=f"o{cc}",
                                     name=f"ps_fc2_{t0}_{cc}")
                           for cc in range(CC)]
                    for kc in range(HC):
                        w_sb = w2pool.tile([128, C], BF16, tag="w2row")
                        nc.sync.dma_start(w_sb, w2[kc * 128:(kc + 1) * 128, :])
                        for cc in range(CC):
                            nc.tensor.matmul(pss[cc][:, :tlen],
                                             w_sb[:, cc * 128:(cc + 1) * 128],
                                             hbuf[:, kc, :tlen],
                                             start=(kc == 0), stop=(kc == HC - 1))
                    for cc in range(CC):
                        ot = outp.tile([128, 394], F32, tag="ot")
                        nc.vector.scalar_tensor_tensor(
                            ot[:, :treal], pss[cc][:, :treal],
                            fc2b_sb[:, cc:cc + 1], x2_fm[:, cc, t0:t0 + treal],
                            op0=OP.add, op1=OP.add)
                        nc.sync.dma_start(
                            out_fm.rearrange("(cc p) t -> p cc t", p=128)
                            [:, cc, t0:t0 + treal], ot[:, :treal])

    nc.compile()
    return nc


# ---------------------------------------------------------------------------
# host side
# ---------------------------------------------------------------------------
_NC1 = None
_NC2 = None


def _get_programs():
    global _NC1, _NC2
    if _NC1 is None:
        _NC1 = build_launch1()
        _NC2 = build_launch2()
    return _NC1, _NC2


def _pre(wT):
    """(Cin, Cout) -> (Cout//128, 128, Cin): [cc, p, kc*128+j] = wT[kc*128+p, cc*128+j]
    so each cc-chunk DMA is fully contiguous."""
    cin, cout = wT.shape
    arr = wT.reshape(cin // 128, 128, cout // 128, 128)
    return np.ascontiguousarray(arr.transpose(2, 1, 0, 3).reshape(
        cout // 128, 128, cin))


def _pre_v(wva):
    """(C, V66) -> (128, CC*V66): [p, kc*V66 + j] = wva[kc*128+p, j]."""
    arr = wva.reshape(CC, 128, V66)
    return np.ascontiguousarray(arr.transpose(1, 0, 2).reshape(128, CC * V66))


def _aug_v(wv_t, vb):
    """wv_t: (C, C) = wv.T -> (C, 1040) with ones-slot col per head zeroed;
    vb -> (1040,) with 1.0 in the ones-slot."""
    wva = np.zeros((C, V66), np.float32)
    vba = np.zeros((V66,), np.float32)
    for hh in range(H):
        wva[:, hh * VW:hh * VW + 64] = wv_t[:, hh * 64:(hh + 1) * 64]
        vba[hh * VW:hh * VW + 64] = vb[hh * 64:(hh + 1) * 64]
        vba[hh * VW + 64] = 1.0
    return wva, vba


def kernel(x, cls_token, tln_g, tln_b, t_wqkv, t_qb, t_vb, t_wproj, t_bproj,
           t_fc_w, t_fc_b, n1_g, n1_b, s_wqkv, s_qb, s_vb, s_wproj, s_bproj,
           n2_g, n2_b, fc1_w, fc1_b, fc2_w, fc2_b):
    x = np.asarray(x, np.float32)
    nc1, nc2 = _get_programs()
    f32 = np.float32
    cont = np.ascontiguousarray

    ident = np.eye(128, dtype=f32)
    mask = np.zeros((128, 128), f32)
    for g in range(16):
        mask[g * 8:(g + 1) * 8, g * 8:(g + 1) * 8] = 1.0

    def cols(v):  # (k*128,) -> (128, k)
        v = np.asarray(v, f32)
        return v.reshape(-1, 128).T

    # launch 1 prep
    twq_T = _pre((np.asarray(t_wqkv)[0:C] * SCALE).T.astype(f32))
    twk_T = _pre(np.asarray(t_wqkv)[C:2 * C].T.astype(f32))
    twv_T = cont(np.asarray(t_wqkv)[2 * C:3 * C].T.astype(f32))
    twv_aug, tvb_aug = _aug_v(twv_T, np.asarray(t_vb, f32))
    twv_aug = _pre_v(twv_aug)
    twp_T = _pre(np.asarray(t_wproj).T.astype(f32))
    tfc_T = _pre(np.asarray(t_fc_w).T.astype(f32))
    tqb_s = (np.asarray(t_qb, f32) * SCALE).astype(f32)
    cb1 = np.concatenate([
        ident, mask, np.broadcast_to(tvb_aug, (128, V66)),
        cols(tln_g), cols(tln_b), cols(tqb_s), cols(t_bproj), cols(t_fc_b),
    ], axis=1).astype(f32)
    cb1 = np.ascontiguousarray(cb1)

    # x in (n, t) token order per (b, h): x[b, t, h*98+nl, :] at row nl*8+t
    xp = np.transpose(x, (0, 2, 1, 3))          # (B, N, T, C)
    in_maps1 = []
    for core in range(N_CORES):
        b, h = core // 2, core % 2
        xtm = cont(xp[b, h * NH:(h + 1) * NH].reshape(TOK1, C))
        in_maps1.append({
            "x_tm": xtm, "x_fm": cont(xtm.T),
            "wq_T": twq_T, "wk_T": twk_T, "wv_T": twv_aug,
            "wp_T": twp_T, "wfc_T": tfc_T, "cblock": cb1,
        })
    res1 = run_bass_kernel_spmd(nc1, in_maps1, core_ids=list(range(N_CORES)))

    # xt_global[b, n*8+t, :] from per-core feature-major outputs
    xt_g = np.empty((B, N * T, C), f32)
    for core in range(N_CORES):
        b, h = core // 2, core % 2
        arr = res1.results[core]["xt_out"]      # (C, 784)
        xt_g[b, h * NH * T:(h + 1) * NH * T] = arr.T

    # launch 2 prep
    swq_T = _pre((np.asarray(s_wqkv)[0:C] * SCALE).T.astype(f32))
    swk_T = _pre(np.asarray(s_wqkv)[C:2 * C].T.astype(f32))
    swv_T = cont(np.asarray(s_wqkv)[2 * C:3 * C].T.astype(f32))
    swv_aug, svb_aug = _aug_v(swv_T, np.asarray(s_vb, f32))
    swv_aug = _pre_v(swv_aug)
    swp_T = _pre(np.asarray(s_wproj).T.astype(f32))
    fc1_T = _pre(np.asarray(fc1_w).T.astype(f32))
    fc2_T = cont(np.asarray(fc2_w).T.astype(ml_dtypes.bfloat16))
    sqb_s = (np.asarray(s_qb, f32) * SCALE).astype(f32)
    cls_np = np.asarray(cls_token, f32)
    cb2 = np.concatenate([
        ident, np.broadcast_to(svb_aug, (128, V66)),
        cols(n1_g), cols(n1_b), cols(n2_g), cols(n2_b), cols(sqb_s),
        cols(s_bproj), cols(fc2_b), cols(fc1_b),
    ], axis=1).astype(f32)
    cb2 = np.ascontiguousarray(cb2)

    in_maps2 = []
    for core in range(N_CORES):
        b, h = core // 2, core % 2
        # rows: [cls_b] + xt[b, n*8 + (4h+fl)] for fl, n  (frame-major)
        xt_b = xt_g[b].reshape(N, T, C)
        rows = xt_b[:, 4 * h:4 * h + 4].transpose(1, 0, 2).reshape(FPC * N, C)
        xstm = np.concatenate([cls_np[b:b + 1], rows], axis=0)
        in_maps2.append({
            "xs_tm": cont(xstm), "res_fm": cont(xstm.T),
            "swq_T": swq_T, "swk_T": swk_T, "swv_T": swv_aug, "swp_T": swp_T,
            "fc1_T": fc1_T, "fc2_T": fc2_T, "cblock": cb2,
        })
    res2 = run_bass_kernel_spmd(nc2, in_maps2, core_ids=list(range(N_CORES)))

    xo = np.empty((B, T, N, C), f32)
    cls_out = np.empty((B, C), f32)
    for core in range(N_CORES):
        b, h = core // 2, core % 2
        arr = res2.results[core]["out_fm"]      # (C, 785)
        body = arr[:, 1:].reshape(C, FPC, N)
        for fl in range(FPC):
            xo[b, 4 * h + fl] = body[:, fl, :].T
        if h == 0:
            cls_out[b] = arr[:, 0]
    return xo, cls_out


# revision 22
# speedup vs baseline: 21311.4627x; 1.0303x over previous
"""TimeSformer-style divided space-time attention block on 8 trn2 NeuronCores.

Sharding: core = 2*b + h (b in 0..3, h in 0..1).
  Launch 1 (temporal attn + fc + residual): core handles batch b, patch-half h
    (98 of 196 patches), all T=8 frames. 784 tokens/core in (n_loc, t) order.
  Launch 2 (spatial attn + cls + MLP):      core handles batch b, frame-half h
    (4 of 8 frames), all 196 patches + cls. Host reshuffles between launches.

Layouts: activations feature-major (C on partitions) for matmuls; LN/softmax
stats token-major via PE transposes; weights host-pre-transposed; q-scale
folded into wq/qb; softmax denominator via ones-column appended to V.
Matmuls in float32r (except fc2: bf16), everything else fp32, exact erf-GELU.
"""

import numpy as np
import ml_dtypes
from contextlib import ExitStack

import concourse.bass as bass
import concourse.tile as tile
from concourse import bacc, mybir
from concourse.bass_utils import run_bass_kernel_spmd

F32 = mybir.dt.float32
F32R = mybir.dt.float32r
BF16 = mybir.dt.bfloat16
AF = mybir.ActivationFunctionType
OP = mybir.AluOpType

B, T, N, C = 4, 8, 196, 1024
H, D = 16, 64
SCALE = D ** -0.5
HID = 4 * C
EPS = 1e-5
NH = N // 2            # 98 patches per core in launch 1
TOK1 = NH * T          # 784 tokens per core, launch 1
FPC = T // 2           # 4 frames per core in launch 2
TOK2 = 1 + FPC * N     # 785 tokens per core, launch 2 (cls + 4*196)
FLAT2 = FPC * 197      # 788: spatial qkv token list, cls duplicated per frame
CC = C // 128          # 8 chunks of C
HC = HID // 128        # 32 chunks of HID
VW = 66                # per-head V block: 64 V + ones col + pad (even N)
V66 = H * VW           # 1056
N_CORES = 8


def _tiles(total, step=128):
    return [(i, min(step, total - i)) for i in range(0, total, step)]


def _bcast_row(ap_1d, parts=128):
    """DRAM (n,) -> DMA-source AP broadcasting over `parts` partitions."""
    return bass.AP(tensor=ap_1d.tensor, offset=ap_1d.offset,
                   ap=[[0, parts]] + list(ap_1d.ap))


def _ln_to_fm(nc, tm_src_tile, tn, dst_fm, dst_cols, g_sb, b_sb,
              eps_sb, psT, tmp, ident_sb):
    """LN a token-major tile (tn<=128 tokens x C) and write normalized*g+b
    transposed into dst_fm[:, cc, ...] (feature-major, fp32r).
    dst_cols: list of (col0, length, src0) runs."""
    st = tmp.tile([128, 2, 6], F32, tag="st")
    nc.vector.bn_stats(st[:tn, 0], tm_src_tile[:tn, 0:512])
    nc.vector.bn_stats(st[:tn, 1], tm_src_tile[:tn, 512:1024])
    mv = tmp.tile([128, 2], F32, tag="mv")
    nc.vector.bn_aggr(mv[:tn], st[:tn])
    rstd = tmp.tile([128, 1], F32, tag="rstd")
    nc.scalar.activation(rstd[:tn], mv[:tn, 1:2], AF.Sqrt, bias=eps_sb[:tn])
    nc.vector.reciprocal(rstd[:tn], rstd[:tn])
    y = tmp.tile([128, C], F32, tag="y")
    nc.vector.tensor_scalar(y[:tn], tm_src_tile[:tn], mv[:tn, 0:1], rstd[:tn],
                            op0=OP.subtract, op1=OP.mult)
    for cc in range(CC):
        pt = psT.tile([128, 128], F32)
        nc.tensor.transpose(pt[:, :tn], y[:tn, cc * 128:(cc + 1) * 128],
                            ident_sb[:tn, :tn])
        for (col0, length, src0) in dst_cols:
            nc.scalar.activation(dst_fm[:, cc, col0:col0 + length],
                                 pt[:, src0:src0 + length], AF.Identity,
                                 bias=b_sb[:, cc:cc + 1], scale=g_sb[:, cc:cc + 1])


def build_launch1():
    nc = bacc.Bacc("TRN2", target_bir_lowering=False, debug=False,
                   enable_asserts=False, num_devices=N_CORES)

    def din(name, shape, dt=F32R):
        return nc.dram_tensor(name, shape, dt, kind="ExternalInput").ap()

    x_tm = din("x_tm", [TOK1, C], F32)
    x_fm = din("x_fm", [C, TOK1], F32)
    wq = din("wq_T", [CC, 128, C]); wk = din("wk_T", [CC, 128, C])
    wv = din("wv_T", [128, CC * V66])
    wp = din("wp_T", [CC, 128, C]); wfc = din("wfc_T", [CC, 128, C])
    # cblock: [0:128 ident][128:256 mask][256:1312 vb_bcast]
    #         [1312.. g,b,qb,bp,fcb cols (8 each)]
    cblock = din("cblock", [128, 1352], F32)
    xt_out = nc.dram_tensor("xt_out", [C, TOK1], F32, kind="ExternalOutput").ap()

    toks = _tiles(TOK1)  # 7 tiles: 6x128 + 16

    with tile.TileContext(nc) as tc, ExitStack() as ctx:
        const = ctx.enter_context(tc.tile_pool(name="const", bufs=1))
        cb = const.tile([128, 1352], F32)
        nc.sync.dma_start(cb, cblock)
        ident_sb = cb[:, 0:128]; mask_sb = cb[:, 128:256]
        vb_sb = cb[:, 256:256 + V66]
        g_sb = cb[:, 1312:1320]; b_sb = cb[:, 1320:1328]
        qb_sb = cb[:, 1328:1336]; bp_sb = cb[:, 1336:1344]
        fcb_sb = cb[:, 1344:1352]
        eps_sb = const.tile([128, 1], F32); nc.vector.memset(eps_sb, EPS)

        big = ctx.enter_context(tc.tile_pool(name="big", bufs=1))
        wpool = ctx.enter_context(tc.tile_pool(name="w", bufs=5))
        def load_w1024(wT, cc):
            w_sb = wpool.tile([128, CC, 128], F32R, tag="w1024")
            nc.sync.dma_start(w_sb, wT[cc].rearrange("p (kc j) -> p kc j", j=128))
            return w_sb

        # -------- stage A: LN + transpose -> x_lnT --------
        x_lnT = big.tile([128, CC, TOK1], F32R, tag="x_lnT")
        with tc.tile_pool(name="ln_tmp", bufs=3) as tmp, \
             tc.tile_pool(name="psA", bufs=4, space="PSUM") as psT:
            for (t0, tn) in toks:
                xt_t = tmp.tile([128, C], F32, tag="xt")
                nc.sync.dma_start(xt_t[:tn], x_tm[t0:t0 + tn])
                _ln_to_fm(nc, xt_t, tn, x_lnT, [(t0, tn, 0)],
                          g_sb, b_sb, eps_sb, psT, tmp, ident_sb)


        # -------- stage B: QKV --------
        q_fm = big.tile([128, CC, TOK1], F32R, tag="q")
        k_fm = big.tile([128, CC, TOK1], F32R, tag="k")
        with tc.tile_pool(name="psB", bufs=4, space="PSUM") as psQ:
            for (dst, wT, bias_cols) in ((q_fm, wq, qb_sb), (k_fm, wk, None)):
                for cc in range(CC):
                    w_sb = load_w1024(wT, cc)
                    for (t0, tlen) in ((0, 392), (392, 392)):
                        ps = psQ.tile([128, 392], F32)
                        for kc in range(CC):
                            nc.tensor.matmul(ps, w_sb[:, kc],
                                             x_lnT[:, kc, t0:t0 + tlen],
                                             start=(kc == 0), stop=(kc == CC - 1))
                        if bias_cols is None:
                            nc.vector.tensor_copy(dst[:, cc, t0:t0 + tlen], ps)
                        else:
                            nc.scalar.activation(dst[:, cc, t0:t0 + tlen], ps,
                                                 AF.Identity,
                                                 bias=bias_cols[:, cc:cc + 1])

        wv_sb = big.tile([128, CC, V66], F32R, tag="wv")
        nc.sync.dma_start(wv_sb, wv.rearrange("p (kc j) -> p kc j", j=V66))
        v_tm = big.tile([128, len(toks), V66], F32R, tag="v")
        with tc.tile_pool(name="psV", bufs=4, space="PSUM") as psV:
            for it, (t0, tn) in enumerate(toks):
                for (n0, nlen) in ((0, 512), (512, 512), (1024, 32)):
                    ps = psV.tile([128, 512], F32)
                    for kc in range(CC):
                        nc.tensor.matmul(ps[:tn, :nlen],
                                         x_lnT[:, kc, t0:t0 + tn],
                                         wv_sb[:, kc, n0:n0 + nlen],
                                         start=(kc == 0), stop=(kc == CC - 1))
                    nc.vector.tensor_add(v_tm[:tn, it, n0:n0 + nlen],
                                         ps[:tn, :nlen], vb_sb[:tn, n0:n0 + nlen])

        # -------- stage C+D: temporal attention + per-tile O transpose ------
        o_tm = big.tile([128, len(toks), C], F32, tag="wv")  # reuses wv slot
        o_fm = big.tile([128, CC, TOK1], F32R, tag="x_lnT")  # reuses slot
        with tc.tile_pool(name="att", bufs=6) as att, \
             tc.tile_pool(name="psL", bufs=3, space="PSUM") as psL, \
             tc.tile_pool(name="psO", bufs=3, space="PSUM") as psO:
            for it, (t0, tn) in enumerate(toks):
                qn = min(256, TOK1 - t0)
                for hh in range(H):
                    ch = hh // 2; po = 64 * (hh % 2)
                    psl = psL.tile([128, 256], F32)
                    nc.tensor.matmul(psl[:tn, :qn],
                                     k_fm[po:po + 64, ch, t0:t0 + tn],
                                     q_fm[po:po + 64, ch, t0:t0 + qn],
                                     start=True, stop=True)
                    et = att.tile([128, 128], F32, tag="et")
                    nc.scalar.activation(et[:tn, :tn], psl[:tn, 0:tn], AF.Exp)
                    at = att.tile([128, 128], F32R, tag="at")
                    nc.vector.tensor_mul(at[:tn, :tn], et[:tn, :tn],
                                         mask_sb[:tn, :tn])
                    pso = psO.tile([128, 66], F32)
                    nc.tensor.matmul(pso[:tn], at[:tn, :tn],
                                     v_tm[:tn, it, hh * VW:hh * VW + 66],
                                     start=True, stop=True)
                    rt = att.tile([128, 1], F32, tag="rt")
                    nc.vector.reciprocal(rt[:tn], pso[:tn, 64:65])
                    nc.vector.tensor_scalar_mul(
                        o_tm[:tn, it, hh * 64:(hh + 1) * 64],
                        pso[:tn, 0:64], rt[:tn])
                with tc.tile_pool(name=f"psD{it}", bufs=2, space="PSUM") as psD:
                    for cc in range(CC):
                        pt = psD.tile([128, 128], F32)
                        nc.tensor.transpose(pt[:, :tn],
                                            o_tm[:tn, it, cc * 128:(cc + 1) * 128],
                                            ident_sb[:tn, :tn])
                        nc.vector.tensor_copy(o_fm[:, cc, t0:t0 + tn],
                                              pt[:, :tn])

        # -------- stage E: proj --------
        p_fm = big.tile([128, CC, TOK1], F32R, tag="q")  # reuses q slot
        with tc.tile_pool(name="psE", bufs=4, space="PSUM") as psE:
            for cc in range(CC):
                w_sb = load_w1024(wp, cc)
                for (t0, tlen) in ((0, 392), (392, 392)):
                    ps = psE.tile([128, 392], F32)
                    for kc in range(CC):
                        nc.tensor.matmul(ps, w_sb[:, kc],
                                         o_fm[:, kc, t0:t0 + tlen],
                                         start=(kc == 0), stop=(kc == CC - 1))
                    nc.scalar.activation(p_fm[:, cc, t0:t0 + tlen], ps,
                                         AF.Identity, bias=bp_sb[:, cc:cc + 1])

        # -------- stage F: t_fc + residual --------
        x_fm_sb = big.tile([128, CC, TOK1], F32, tag="v")  # reuses v slot
        nc.sync.dma_start(x_fm_sb, x_fm.rearrange("(cc p) t -> p cc t", p=128))
        xt_fm = big.tile([128, CC, TOK1], F32, tag="k")  # reuses k slot
        with tc.tile_pool(name="psF", bufs=4, space="PSUM") as psF:
            for cc in range(CC):
                w_sb = load_w1024(wfc, cc)
                for (t0, tlen) in ((0, 392), (392, 392)):
                    ps = psF.tile([128, 392], F32)
                    for kc in range(CC):
                        nc.tensor.matmul(ps, w_sb[:, kc],
                                         p_fm[:, kc, t0:t0 + tlen],
                                         start=(kc == 0), stop=(kc == CC - 1))
                    nc.vector.scalar_tensor_tensor(
                        xt_fm[:, cc, t0:t0 + tlen], ps, fcb_sb[:, cc:cc + 1],
                        x_fm_sb[:, cc, t0:t0 + tlen], op0=OP.add, op1=OP.add)
                    nc.sync.dma_start(
                        xt_out.rearrange("(cc p) t -> p cc t", p=128)
                        [:, cc, t0:t0 + tlen], xt_fm[:, cc, t0:t0 + tlen])

    nc.compile()
    return nc


def build_launch2(use_collective=True):
    nc = bacc.Bacc("TRN2", target_bir_lowering=False, debug=False,
                   enable_asserts=False, num_devices=N_CORES)

    def din(name, shape, dt=F32R):
        return nc.dram_tensor(name, shape, dt, kind="ExternalInput").ap()

    xs_tm = din("xs_tm", [TOK2, C], F32)
    res_in = din("res_fm", [C, TOK2], F32)
    wq = din("swq_T", [CC, 128, C]); wk = din("swk_T", [CC, 128, C])
    wv = din("swv_T", [128, CC * V66])
    wp = din("swp_T", [CC, 128, C])
    w1 = din("fc1_T", [HC, 128, C]); w2 = din("fc2_T", [HID, C], BF16)
    # cblock: [0:128 ident][128:1184 vb][1184.. g1,b1,g2,b2,qb,bp,fc2b (8 each)]
    #         [1240:1272 fc1b]
    cblock = din("cblock", [128, 1272], F32)
    out_fm = nc.dram_tensor("out_fm", [C, TOK2], F32, kind="ExternalOutput").ap()
    cls_bnc = nc.dram_tensor("cls_bnc", [128, CC], F32, kind="Internal").ap()
    clsm_bnc = nc.dram_tensor("clsm_bnc", [128, CC], F32, kind="Internal").ap()

    toks2 = _tiles(TOK2)   # 7 tiles: 6x128 + 17

    # token-major index i (0..784) -> spatial flat col (cls dup'd per frame):
    #   i==0 -> cls (cols 197*f); i>=1: f=(i-1)//196, n=(i-1)%196 -> 197f+1+n
    def fm_runs(t0, tn):
        runs = []
        i = max(t0, 1)
        while i < t0 + tn:
            f = (i - 1) // 196
            end = min(t0 + tn, 1 + (f + 1) * 196)
            runs.append((197 * f + 1 + ((i - 1) % 196), end - i, i - t0))
            i = end
        return runs

    with tile.TileContext(nc) as tc, ExitStack() as ctx:
        const = ctx.enter_context(tc.tile_pool(name="const", bufs=1))
        cb = const.tile([128, 1272], F32)
        nc.sync.dma_start(cb, cblock)
        ident_sb = cb[:, 0:128]
        vb_sb = cb[:, 128:128 + V66]
        g1_sb = cb[:, 1184:1192]; b1_sb = cb[:, 1192:1200]
        g2_sb = cb[:, 1200:1208]; b2_sb = cb[:, 1208:1216]
        qb_sb = cb[:, 1216:1224]; bp_sb = cb[:, 1224:1232]
        fc2b_sb = cb[:, 1232:1240]
        fc1b_sb = cb[:, 1240:1272]
        eps_sb = const.tile([128, 1], F32); nc.vector.memset(eps_sb, EPS)
        cls_m = const.tile([128, CC], F32, tag="cls_m")
        cls_part = const.tile([128, CC], F32, tag="cls_part")

        big = ctx.enter_context(tc.tile_pool(name="big", bufs=1))
        wpool = ctx.enter_context(tc.tile_pool(name="w", bufs=3))
        def load_w1024(wT, cc):
            w_sb = wpool.tile([128, CC, 128], F32R, tag="w1024")
            nc.sync.dma_start(w_sb, wT[cc].rearrange("p (kc j) -> p kc j", j=128))
            return w_sb

        # -------- stage A: LN1 -> x_lnT (flat 4x197, cls duplicated) --------
        x_lnT = big.tile([128, CC, FLAT2], F32R, tag="x_lnT")
        with tc.tile_pool(name="ln_tmp", bufs=2) as tmp, \
             tc.tile_pool(name="psA", bufs=4, space="PSUM") as psT:
            for (t0, tn) in toks2:
                xt_t = tmp.tile([128, C], F32, tag="xt")
                nc.sync.dma_start(xt_t[:tn], xs_tm[t0:t0 + tn])
                runs = fm_runs(t0, tn)
                if t0 == 0:  # cls goes to col 197*f of every frame
                    runs = runs + [(197 * f, 1, 0) for f in range(FPC)]
                _ln_to_fm(nc, xt_t, tn, x_lnT, runs,
                          g1_sb, b1_sb, eps_sb, psT, tmp, ident_sb)
        res_sb = big.tile([128, CC, TOK2], F32, tag="hbuf")
        nc.sync.dma_start(res_sb, res_in.rearrange("(cc p) t -> p cc t", p=128))

        # -------- stage B: QKV --------
        wv_sb = big.tile([128, CC, V66], F32R, tag="wv")
        nc.sync.dma_start(wv_sb, wv.rearrange("p (kc j) -> p kc j", j=V66))
        v_tm = big.tile([128, FPC, 2, V66], F32R, tag="v")
        with tc.tile_pool(name="psV", bufs=4, space="PSUM") as psV:
            for f in range(FPC):
                for ki, (k0, kn) in enumerate(((0, 128), (128, 69))):
                    for (n0, nlen) in ((0, 512), (512, 512), (1024, 32)):
                        ps = psV.tile([128, 512], F32)
                        for kc in range(CC):
                            nc.tensor.matmul(
                                ps[:kn, :nlen],
                                x_lnT[:, kc, 197 * f + k0:197 * f + k0 + kn],
                                wv_sb[:, kc, n0:n0 + nlen],
                                start=(kc == 0), stop=(kc == CC - 1))
                        nc.vector.tensor_add(v_tm[:kn, f, ki, n0:n0 + nlen],
                                             ps[:kn, :nlen],
                                             vb_sb[:kn, n0:n0 + nlen])

        # q_fm padded to 256 per frame so logits matmuls get N=256 (fp32r fast)
        q_fm = big.tile([128, CC, FPC, 256], F32R, tag="q")
        for cc in range(CC):
            for f in range(FPC):
                nc.vector.memset(q_fm.bitcast(F32)[:, cc, f, 197:256], 0.0)
        k_fm = big.tile([128, CC, FLAT2], F32R, tag="k")
        halves = ((0, 394), (394, 394))  # frame-boundary-aligned halves of 788
        with tc.tile_pool(name="psB", bufs=4, space="PSUM") as psQ:
            for cc in range(CC):
                w_sb = load_w1024(wq, cc)
                for hi, (t0, tlen) in enumerate(halves):
                    ps = psQ.tile([128, 394], F32)
                    for kc in range(CC):
                        nc.tensor.matmul(ps, w_sb[:, kc],
                                         x_lnT[:, kc, t0:t0 + tlen],
                                         start=(kc == 0), stop=(kc == CC - 1))
                    for fo in range(2):  # two frames per half
                        f = 2 * hi + fo
                        nc.scalar.activation(q_fm[:, cc, f, 0:197],
                                             ps[:, 197 * fo:197 * fo + 197],
                                             AF.Identity, bias=qb_sb[:, cc:cc + 1])
            for cc in range(CC):
                w_sb = load_w1024(wk, cc)
                for (t0, tlen) in halves:
                    ps = psQ.tile([128, 394], F32)
                    for kc in range(CC):
                        nc.tensor.matmul(ps, w_sb[:, kc],
                                         x_lnT[:, kc, t0:t0 + tlen],
                                         start=(kc == 0), stop=(kc == CC - 1))
                    nc.vector.tensor_copy(k_fm[:, cc, t0:t0 + tlen], ps)

        # -------- stage C+D: spatial attention + per-frame O transpose -----
        o_tm = big.tile([128, FPC, 2, C], F32, tag="wv")  # reuses wv slot
        o_fm = big.tile([128, CC, FLAT2], F32R, tag="x_lnT")  # reuses slot
        with tc.tile_pool(name="att", bufs=4) as att, \
             tc.tile_pool(name="psL", bufs=3, space="PSUM") as psL, \
             tc.tile_pool(name="psO", bufs=3, space="PSUM") as psO:
            for f in range(FPC):
                for hh in range(H):
                    ch = hh // 2; po = 64 * (hh % 2)
                    a_t = []
                    for ki, (k0, kn) in enumerate(((0, 128), (128, 69))):
                        psl = psL.tile([128, 256], F32)
                        nc.tensor.matmul(
                            psl[:kn, :],
                            k_fm[po:po + 64, ch, 197 * f + k0:197 * f + k0 + kn],
                            q_fm[po:po + 64, ch, f, 0:256],
                            start=True, stop=True)
                        at = att.tile([128, 197], F32R, tag=f"at{ki}")
                        nc.scalar.activation(at[:kn, :], psl[:kn, 0:197], AF.Exp)
                        a_t.append(at)
                    for qi, (q0, qlen) in enumerate(((0, 128), (128, 69))):
                        pso = psO.tile([128, 66], F32)
                        nc.tensor.matmul(pso[:qlen], a_t[0][:, q0:q0 + qlen],
                                         v_tm[:, f, 0, hh * VW:hh * VW + 66],
                                         start=True, stop=False)
                        nc.tensor.matmul(pso[:qlen], a_t[1][:69, q0:q0 + qlen],
                                         v_tm[:69, f, 1, hh * VW:hh * VW + 66],
                                         start=False, stop=True)
                        rt = att.tile([128, 1], F32, tag="rt")
                        nc.vector.reciprocal(rt[:qlen], pso[:qlen, 64:65])
                        nc.vector.tensor_scalar_mul(
                            o_tm[:qlen, f, qi, hh * 64:(hh + 1) * 64],
                            pso[:qlen, 0:64], rt[:qlen])
                with tc.tile_pool(name=f"psD{f}", bufs=2, space="PSUM") as psD:
                    for qi, (q0, qlen) in enumerate(((0, 128), (128, 69))):
                        for cc in range(CC):
                            pt = psD.tile([128, 128], F32)
                            nc.tensor.transpose(pt[:, :qlen],
                                                o_tm[:qlen, f, qi,
                                                     cc * 128:(cc + 1) * 128],
                                                ident_sb[:qlen, :qlen])
                            nc.vector.tensor_copy(
                                o_fm[:, cc, 197 * f + q0:197 * f + q0 + qlen],
                                pt[:, :qlen])

        # -------- stage E: proj -> p_fm (f32) --------
        p_fm = big.tile([128, CC, FLAT2], F32, tag="q")  # reuses q slot
        with tc.tile_pool(name="psE", bufs=4, space="PSUM") as psE:
            for cc in range(CC):
                w_sb = load_w1024(wp, cc)
                for (t0, tlen) in halves:
                    ps = psE.tile([128, 394], F32)
                    for kc in range(CC):
                        nc.tensor.matmul(ps, w_sb[:, kc],
                                         o_fm[:, kc, t0:t0 + tlen],
                                         start=(kc == 0), stop=(kc == CC - 1))
                    nc.scalar.activation(p_fm[:, cc, t0:t0 + tlen], ps,
                                         AF.Identity, bias=bp_sb[:, cc:cc + 1])

        # -------- stage F: cls partial mean + AllReduce over the pair -------
        cls_view = bass.AP(tensor=p_fm.tensor, offset=p_fm.offset,
                           ap=[list(p_fm.ap[0]), [FLAT2, CC], [197, FPC]])
        nc.vector.tensor_reduce(cls_part, cls_view, axis=mybir.AxisListType.X,
                                op=OP.add)
        nc.scalar.mul(cls_part, cls_part, 1.0 / T)
        nc.sync.dma_start(cls_bnc, cls_part)
        if use_collective:
            nc.gpsimd.collective_compute(
                "AllReduce", OP.add,
                replica_groups=[[0, 1], [2, 3], [4, 5], [6, 7]],
                ins=[cls_bnc], outs=[clsm_bnc])
        else:  # cost-model sim: collective unsupported; timing-equivalent copy
            nc.sync.dma_start(clsm_bnc, cls_bnc)
        nc.sync.dma_start(cls_m, clsm_bnc)

        # -------- stage G: x2 = res + [cls_m | xs] --------
        x2_fm = big.tile([128, CC, TOK2], F32, tag="k")  # reuses k slot
        for cc in range(CC):
            nc.vector.tensor_scalar_add(x2_fm[:, cc, 0:1], res_sb[:, cc, 0:1],
                                        cls_m[:, cc:cc + 1])
            for f in range(FPC):
                nc.vector.tensor_add(
                    x2_fm[:, cc, 1 + 196 * f:1 + 196 * (f + 1)],
                    res_sb[:, cc, 1 + 196 * f:1 + 196 * (f + 1)],
                    p_fm[:, cc, 197 * f + 1:197 * f + 197])

        # -------- stage H: LN2 (fm -> tm -> fm) --------
        x2_lnT = big.tile([128, CC, TOK2 + 1], F32R, tag="v")  # reuses v slot
        for cc in range(CC):
            nc.vector.memset(x2_lnT.bitcast(F32)[:, cc, TOK2:TOK2 + 1], 0.0)
        with tc.tile_pool(name="ln2_tmp", bufs=2) as tmp, \
             tc.tile_pool(name="psH", bufs=6, space="PSUM") as psT:
            for (t0, tn) in toks2[3:] + toks2[1:3] + toks2[:1]:
                x2_t = tmp.tile([128, C], F32, tag="xt")
                for cc in range(CC):
                    pt = psT.tile([128, 128], F32)
                    nc.tensor.transpose(pt[:tn, :], x2_fm[:, cc, t0:t0 + tn],
                                        ident_sb)
                    eng = nc.vector.tensor_copy if cc % 2 else nc.scalar.copy
                    eng(x2_t[:tn, cc * 128:(cc + 1) * 128], pt[:tn, :])
                _ln_to_fm(nc, x2_t, tn, x2_lnT, [(t0, tn, 0)],
                          g2_sb, b2_sb, eps_sb, psT, tmp, ident_sb)

        # -------- stage I: MLP + residual --------
        mhalves = ((394, 392, 391), (0, 394, 394))  # (t0, mm_N, real)
        hbuf = big.tile([128, HC, 394], BF16, tag="hbuf")
        with tc.tile_pool(name="out_t", bufs=3) as outp:
            for (t0, tlen, treal) in mhalves:
                with tc.tile_pool(name=f"psI{t0}", bufs=4, space="PSUM") as psM:
                    for cc in range(HC):
                        w_sb = load_w1024(w1, cc)
                        ps = psM.tile([128, 394], F32)
                        for kc in range(CC):
                            nc.tensor.matmul(ps[:, :tlen], w_sb[:, kc],
                                             x2_lnT[:, kc, t0:t0 + tlen],
                                             start=(kc == 0), stop=(kc == CC - 1))
                        nc.scalar.activation(hbuf[:, cc, :tlen], ps[:, :tlen],
                                             AF.Gelu, bias=fc1b_sb[:, cc:cc + 1])
                with tc.tile_pool(name=f"psI2{t0}", bufs=1, space="PSUM") as psM2, \
                     tc.tile_pool(name=f"w2p{t0}", bufs=3) as w2pool:
                    pss = [psM2.tile([128, 394], F32, tag=f"o{cc}",
                                     name=f"ps_fc2_{t0}_{cc}")
                           for cc in range(CC)]
                    for kc in range(HC):
                        w_sb = w2pool.tile([128, C], BF16, tag="w2row")
                        nc.sync.dma_start(w_sb, w2[kc * 128:(kc + 1) * 128, :])
                        for cc in range(CC):
                            nc.tensor.matmul(pss[cc][:, :tlen],
                                             w_sb[:, cc * 128:(cc + 1) * 128],
                                             hbuf[:, kc, :tlen],
                                             start=(kc == 0), stop=(kc == HC - 1))
                    for cc in range(CC):
                        ot = outp.tile([128, 394], F32, tag="ot")
                        nc.vector.scalar_tensor_tensor(
                            ot[:, :treal], pss[cc][:, :treal],
                            fc2b_sb[:, cc:cc + 1], x2_fm[:, cc, t0:t0 + treal],
                            op0=OP.add, op1=OP.add)
                        nc.sync.dma_start(
                            out_fm.rearrange("(cc p) t -> p cc t", p=128)
                            [:, cc, t0:t0 + treal], ot[:, :treal])

    nc.compile()
    return nc


# ---------------------------------------------------------------------------
# host side
# ---------------------------------------------------------------------------
_NC1 = None
_NC2 = None


def _get_programs():
    global _NC1, _NC2
    if _NC1 is None:
        _NC1 = build_launch1()
        _NC2 = build_launch2()
    return _NC1, _NC2


def _pre(wT):
    """(Cin, Cout) -> (Cout//128, 128, Cin): [cc, p, kc*128+j] = wT[kc*128+p, cc*128+j]
    so each cc-chunk DMA is fully contiguous."""
    cin, cout = wT.shape
    arr = wT.reshape(cin // 128, 128, cout // 128, 128)
    return np.ascontiguousarray(arr.transpose(2, 1, 0, 3).reshape(
        cout // 128, 128, cin))


def _pre_v(wva):
    """(C, V66) -> (128, CC*V66): [p, kc*V66 + j] = wva[kc*128+p, j]."""
    arr = wva.reshape(CC, 128, V66)
    return np.ascontiguousarray(arr.transpose(1, 0, 2).reshape(128, CC * V66))


def _aug_v(wv_t, vb):
    """wv_t: (C, C) = wv.T -> (C, 1040) with ones-slot col per head zeroed;
    vb -> (1040,) with 1.0 in the ones-slot."""
    wva = np.zeros((C, V66), np.float32)
    vba = np.zeros((V66,), np.float32)
    for hh in range(H):
        wva[:, hh * VW:hh * VW + 64] = wv_t[:, hh * 64:(hh + 1) * 64]
        vba[hh * VW:hh * VW + 64] = vb[hh * 64:(hh + 1) * 64]
        vba[hh * VW + 64] = 1.0
    return wva, vba


def kernel(x, cls_token, tln_g, tln_b, t_wqkv, t_qb, t_vb, t_wproj, t_bproj,
           t_fc_w, t_fc_b, n1_g, n1_b, s_wqkv, s_qb, s_vb, s_wproj, s_bproj,
           n2_g, n2_b, fc1_w, fc1_b, fc2_w, fc2_b):
    x = np.asarray(x, np.float32)
    nc1, nc2 = _get_programs()
    f32 = np.float32
    cont = np.ascontiguousarray

    ident = np.eye(128, dtype=f32)
    mask = np.zeros((128, 128), f32)
    for g in range(16):
        mask[g * 8:(g + 1) * 8, g * 8:(g + 1) * 8] = 1.0

    def cols(v):  # (k*128,) -> (128, k)
        v = np.asarray(v, f32)
        return v.reshape(-1, 128).T

    # launch 1 prep
    twq_T = _pre((np.asarray(t_wqkv)[0:C] * SCALE).T.astype(f32))
    twk_T = _pre(np.asarray(t_wqkv)[C:2 * C].T.astype(f32))
    twv_T = cont(np.asarray(t_wqkv)[2 * C:3 * C].T.astype(f32))
    twv_aug, tvb_aug = _aug_v(twv_T, np.asarray(t_vb, f32))
    twv_aug = _pre_v(twv_aug)
    twp_T = _pre(np.asarray(t_wproj).T.astype(f32))
    tfc_T = _pre(np.asarray(t_fc_w).T.astype(f32))
    tqb_s = (np.asarray(t_qb, f32) * SCALE).astype(f32)
    cb1 = np.concatenate([
        ident, mask, np.broadcast_to(tvb_aug, (128, V66)),
        cols(tln_g), cols(tln_b), cols(tqb_s), cols(t_bproj), cols(t_fc_b),
    ], axis=1).astype(f32)
    cb1 = np.ascontiguousarray(cb1)

    # x in (n, t) token order per (b, h): x[b, t, h*98+nl, :] at row nl*8+t
    xp = np.transpose(x, (0, 2, 1, 3))          # (B, N, T, C)
    in_maps1 = []
    for core in range(N_CORES):
        b, h = core // 2, core % 2
        xtm = cont(xp[b, h * NH:(h + 1) * NH].reshape(TOK1, C))
        in_maps1.append({
            "x_tm": xtm, "x_fm": cont(xtm.T),
            "wq_T": twq_T, "wk_T": twk_T, "wv_T": twv_aug,
            "wp_T": twp_T, "wfc_T": tfc_T, "cblock": cb1,
        })
    res1 = run_bass_kernel_spmd(nc1, in_maps1, core_ids=list(range(N_CORES)))

    # xt_global[b, n*8+t, :] from per-core feature-major outputs
    xt_g = np.empty((B, N * T, C), f32)
    for core in range(N_CORES):
        b, h = core // 2, core % 2
        arr = res1.results[core]["xt_out"]      # (C, 784)
        xt_g[b, h * NH * T:(h + 1) * NH * T] = arr.T

    # launch 2 prep
    swq_T = _pre((np.asarray(s_wqkv)[0:C] * SCALE).T.astype(f32))
    swk_T = _pre(np.asarray(s_wqkv)[C:2 * C].T.astype(f32))
    swv_T = cont(np.asarray(s_wqkv)[2 * C:3 * C].T.astype(f32))
    swv_aug, svb_aug = _aug_v(swv_T, np.asarray(s_vb, f32))
    swv_aug = _pre_v(swv_aug)
    swp_T = _pre(np.asarray(s_wproj).T.astype(f32))
    fc1_T = _pre(np.asarray(fc1_w).T.astype(f32))
    fc2_T = cont(np.asarray(fc2_w).T.astype(ml_dtypes.bfloat16))
    sqb_s = (np.asarray(s_qb, f32) * SCALE).astype(f32)
    cls_np = np.asarray(cls_token, f32)
    cb2 = np.concatenate([
        ident, np.broadcast_to(svb_aug, (128, V66)),
        cols(n1_g), cols(n1_b), cols(n2_g), cols(n2_b), cols(sqb_s),
        cols(s_bproj), cols(fc2_b), cols(fc1_b),
    ], axis=1).astype(f32)
    cb2 = np.ascontiguousarray(cb2)

    in_maps2 = []
    for core in range(N_CORES):
        b, h = core // 2, core % 2
        # rows: [cls_b] + xt[b, n*8 + (4h+fl)] for fl, n  (frame-major)
        xt_b = xt_g[b].reshape(N, T, C)
        rows = xt_b[:, 4 * h:4 * h + 4].transpose(1, 0, 2).reshape(FPC * N, C)
        xstm = np.concatenate([cls_np[b:b + 1], rows], axis=0)
        in_maps2.append({
            "xs_tm": cont(xstm), "res_fm": cont(xstm.T),
            "swq_T": swq_T, "swk_T": swk_T, "swv_T": swv_aug, "swp_T": swp_T,
            "fc1_T": fc1_T, "fc2_T": fc2_T, "cblock": cb2,
        })
    res2 = run_bass_kernel_spmd(nc2, in_maps2, core_ids=list(range(N_CORES)))

    xo = np.empty((B, T, N, C), f32)
    cls_out = np.empty((B, C), f32)
    for core in range(N_CORES):
        b, h = core // 2, core % 2
        arr = res2.results[core]["out_fm"]      # (C, 785)
        body = arr[:, 1:].reshape(C, FPC, N)
        for fl in range(FPC):
            xo[b, 4 * h + fl] = body[:, fl, :].T
        if h == 0:
            cls_out[b] = arr[:, 0]
    return xo, cls_out
